# revision 1
# baseline (speedup 1.0000x reference)
"""CACombiner Trainium2 kernel: conv-projected efficient attention + FFN.

Data-parallel over batch: 8 batch elements -> 8 NeuronCores, identical SPMD
program per core. All heavy matmuls run as float32r (full PE rate); the
attention-weight path (exp(k), v, softmax(q), ctx) runs in bf16.
"""
import sys
sys.path.insert(0, "/opt/trn_rl_repo")
from contextlib import ExitStack

import numpy as np

import concourse.bass as bass
import concourse.tile as tile
from concourse import mybir, bacc
from concourse.bass_utils import run_bass_kernel_spmd
from concourse.alu_op_type import AluOpType

F32 = mybir.dt.float32
F32R = mybir.dt.float32r
BF16 = mybir.dt.bfloat16
AFT = mybir.ActivationFunctionType
Ax = mybir.AxisListType

B, C, L = 8, 512, 4096
H, DK = 8, 64
EPS = 1e-5
CC = C // 128          # 4 channel chunks
NL1 = L // 128         # 32 phase-1 l-tiles
NL2 = L // 512         # 8 phase-2 l-tiles

_CACHE = {}
LAST_RESULT = None


def _build_program():
    nc = bacc.Bacc("TRN2", target_bir_lowering=False, debug=False)

    def din(name, shape, dtype):
        return nc.dram_tensor(name, list(shape), dtype, kind="ExternalInput").ap()

    z1d = din("z1", (C, L), F32R)
    z2d = din("z2", (C, L), F32R)
    WqTt_d = din("WqTt", (128, CC, 512), F32R)
    bq_row_d = din("bq_row", (1, 512), F32R)
    WkvTt_d = din("WkvTt", (128, CC, 1024), F32R)
    WrTt_d = din("WrTt", (128, CC, 512), F32R)
    W1gTt_d = din("W1gTt", (128, CC, 1024), F32R)
    W2gTt_d = din("W2gTt", (128, 8, 512), F32R)
    U1W_d = din("U1W", (2, 1024), F32R)
    u2ct_d = din("u2ct", (128, 8), F32R)
    G2B_d = din("G2B", (2, 512), F32R)
    ivgt_d = din("ivgt", (128, CC), F32R)
    inv512_d = din("inv512", (128, 1), F32R)
    ones1x128_d = din("ones1x128", (1, 128), F32R)
    ident_d = din("ident", (128, 128), BF16)
    br_c_d = din("br_c", (128, CC), F32)
    bv_c_d = din("bv_c", (128, CC), F32)
    be2_c_d = din("be2_c", (128, CC), F32)
    eps_c_d = din("eps_c", (128, 1), F32)
    ones_row_d = din("ones_row", (1, 512), F32R)
    outd = nc.dram_tensor("out", [C, L], F32, kind="ExternalOutput").ap()

    z1r = z1d.rearrange("(cc p) l -> p cc l", p=128)
    z2r = z2d.rearrange("(cc p) l -> p cc l", p=128)

    mm = nc.tensor.matmul
    tt = nc.vector.tensor_tensor
    ts = nc.vector.tensor_scalar
    stt = nc.vector.scalar_tensor_tensor
    act = nc.scalar.activation

    with tile.TileContext(nc) as tc, ExitStack() as ctx:
        cpool = ctx.enter_context(tc.tile_pool(name="consts", bufs=1))

        def const_tile(shape, dtype, src, tag):
            t = cpool.tile(list(shape), dtype, tag=tag, name=tag)
            nc.sync.dma_start(t[:], src)
            return t

        WqTt = const_tile((128, CC, 512), F32R, WqTt_d, "WqTt")
        bq_row = const_tile((1, 512), F32R, bq_row_d, "bq_row")
        WkvTt = const_tile((128, CC, 1024), F32R, WkvTt_d, "WkvTt")
        WrTt = const_tile((128, CC, 512), F32R, WrTt_d, "WrTt")
        W1gTt = const_tile((128, CC, 1024), F32R, W1gTt_d, "W1gTt")
        W2gTt = const_tile((128, 8, 512), F32R, W2gTt_d, "W2gTt")
        U1W = const_tile((2, 1024), F32R, U1W_d, "U1W")
        u2ct = const_tile((128, 8), F32R, u2ct_d, "u2ct")
        G2B = const_tile((2, 512), F32R, G2B_d, "G2B")
        ivgt = const_tile((128, CC), F32R, ivgt_d, "ivgt")
        inv512 = const_tile((128, 1), F32R, inv512_d, "inv512")
        ones1x128 = const_tile((1, 128), F32R, ones1x128_d, "ones1x128")
        ident = const_tile((128, 128), BF16, ident_d, "ident")
        br_c = const_tile((128, CC), F32, br_c_d, "br_c")
        bv_c = const_tile((128, CC), F32, bv_c_d, "bv_c")
        be2_c = const_tile((128, CC), F32, be2_c_d, "be2_c")
        eps_c = const_tile((128, 1), F32, eps_c_d, "eps_c")
        ones_row = const_tile((1, 512), F32R, ones_row_d, "ones_row")

        # persistent across phases
        qsm = cpool.tile([128, CC, L], BF16, tag="qsm", name="qsm")      # softmaxed q, channels-first
        ctxbd = [cpool.tile([128, 128], BF16, tag=f"ctxbd{p}", name=f"ctxbd{p}") for p in range(CC)]

        # ---------------- Phase 1: q softmax + k/v + ctx accumulation ----------------
        with ExitStack() as p1:
            lp1 = p1.enter_context(tc.tile_pool(name="lp1", bufs=2))
            ps_ctx = p1.enter_context(tc.tile_pool(name="ps_ctx", bufs=1, space="PSUM"))
            ps_w = p1.enter_context(tc.tile_pool(name="ps_w", bufs=1, space="PSUM"))

            ctxps = [ps_ctx.tile([128, 129], F32, tag=f"ctx{p}", name=f"ctxps{p}") for p in range(CC)]

            for lt in range(NL1):
                sl = slice(lt * 128, (lt + 1) * 128)
                z1c = lp1.tile([128, CC, 128], F32R, tag="z1c")
                nc.sync.dma_start(z1c[:], z1r[:, :, sl])
                z2c = lp1.tile([128, CC, 128], F32R, tag="z2c")
                nc.sync.dma_start(z2c[:], z2r[:, :, sl])

                # qT [l,128][o,512] = z1^T Wq^T + bq
                qps = ps_w.tile([128, 512], F32, tag="qps")
                for cc in range(CC):
                    mm(qps[:], z1c[:, cc, :], WqTt[:, cc, :], start=(cc == 0), stop=False)
                mm(qps[:], ones1x128[:], bq_row[:], start=False, stop=True)

                # exp + per-head sums (ACT accumulate), then normalize
                EqT = lp1.tile([128, 512], F32, tag="EqT")
                Sq = lp1.tile([128, 8], F32, tag="Sq")
                for h in range(H):
                    hs = slice(h * 64, (h + 1) * 64)
                    act(EqT[:, hs], qps[:, hs], AFT.Exp, accum_out=Sq[:, h:h + 1])
                rq = lp1.tile([128, 8], F32, tag="rq")
                nc.vector.reciprocal(rq[:], Sq[:])
                qsmT = lp1.tile([128, 512], BF16, tag="qsmT")
                tt(qsmT[:].rearrange("p (g x) -> p g x", x=64),
                   EqT[:].rearrange("p (g x) -> p g x", x=64),
                   rq[:].unsqueeze(2).broadcast_to([128, 8, 64]), AluOpType.mult)

                # transpose qsmT back to channels-first into qsm
                tps = ps_w.tile([128, 512], BF16, tag="tps")
                for cc in range(CC):
                    cs = slice(cc * 128, (cc + 1) * 128)
                    nc.tensor.transpose(tps[:, cs], qsmT[:, cs], ident[:])
                nc.vector.tensor_copy(
                    qsm[:, :, sl],
                    tps[:].rearrange("p (cc x) -> p cc x", x=128))

                # kT | vT
                kvps = ps_w.tile([128, 1024], F32, tag="kvps")
                for cc in range(CC):
                    mm(kvps[:, 0:512], z2c[:, cc, :], WkvTt[:, cc, 0:512],
                       start=(cc == 0), stop=(cc == CC - 1))
                for cc in range(CC):
                    mm(kvps[:, 512:1024], z2c[:, cc, :], WkvTt[:, cc, 512:1024],
                       start=(cc == 0), stop=(cc == CC - 1))
                EkT = lp1.tile([128, 512], BF16, tag="EkT")
                act(EkT[:], kvps[:, 0:512], AFT.Exp)
                vT = lp1.tile([128, 516], BF16, tag="vT")
                nc.vector.tensor_copy(
                    vT[:].rearrange("p (pr x) -> p pr x", pr=4)[:, :, 0:128],
                    kvps[:, 512:1024].rearrange("p (pr x) -> p pr x", pr=4))
                nc.vector.memset(vT[:].rearrange("p (pr x) -> p pr x", pr=4)[:, :, 128:129], 1.0)

                # ctx accumulation: per head-pair [2heads-k, 2heads-v | S]
                for pr in range(CC):
                    mm(ctxps[pr][:], EkT[:, pr * 128:(pr + 1) * 128],
                       vT[:, pr * 129:(pr + 1) * 129],
                       start=(lt == 0), stop=(lt == NL1 - 1), skip_group_check=True)

            # finalize ctx: normalize rows by S, build block-diagonal bf16 tiles
            for pr in range(CC):
                rs = lp1.tile([128, 1], F32, tag="rs")
                nc.vector.reciprocal(rs[:], ctxps[pr][:, 128:129])
                nc.vector.memset(ctxbd[pr][:], 0.0)
                ts(ctxbd[pr][0:64, 0:64], ctxps[pr][0:64, 0:64], rs[0:64, :], None,
                   AluOpType.mult)
                ts(ctxbd[pr][64:128, 64:128], ctxps[pr][64:128, 64:128], rs[64:128, :], None,
                   AluOpType.mult)

        # ---------------- Phase 2: attention apply + reprojection + LN/FFN ----------------
        with ExitStack() as p2:
            lp2 = p2.enter_context(tc.tile_pool(name="lp2", bufs=2))
            lph = p2.enter_context(tc.tile_pool(name="lph", bufs=1))
            ps_big = p2.enter_context(tc.tile_pool(name="ps_big", bufs=5, space="PSUM"))
            ps_row = p2.enter_context(tc.tile_pool(name="ps_row", bufs=2, space="PSUM"))

            for lt in range(NL2):
                sl = slice(lt * 512, (lt + 1) * 512)
                z1res = lp2.tile([128, CC, 512], F32R, tag="z1res", bufs=1)
                nc.sync.dma_start(z1res[:], z1r[:, :, sl])

                # att[v,l] = ctx_bd @ qsm + bv
                att = []
                for pr in range(CC):
                    aps = ps_big.tile([128, 512], F32, tag="big")
                    mm(aps[:], ctxbd[pr][:], qsm[:, pr, sl], start=True, stop=True)
                    a = lph.tile([128, 512], F32R, tag=f"att{pr}")
                    ts(a[:], aps[:], bv_c[:, pr:pr + 1], None, AluOpType.add)
                    att.append(a)

                # z = Wr att + br + z1
                zt = []
                for ot in range(CC):
                    zps = ps_big.tile([128, 512], F32, tag="big")
                    for pr in range(CC):
                        mm(zps[:], WrTt[:, pr, ot * 128:(ot + 1) * 128], att[pr][:],
                           start=(pr == 0), stop=(pr == CC - 1))
                    z = lph.tile([128, 512], F32R, tag=f"z{ot}")
                    stt(z[:], zps[:], br_c[:, ot:ot + 1], z1res[:, ot, :].bitcast(F32),
                        AluOpType.add, AluOpType.add)
                    zt.append(z)

                # LN1 stats rows
                mups = ps_row.tile([1, 512], F32, tag="row")
                for ot in range(CC):
                    mm(mups[:], inv512[:], zt[ot][:], start=(ot == 0), stop=(ot == CC - 1))
                e2ps = ps_row.tile([1, 512], F32, tag="row")
                for ot in range(CC):
                    zsq = lp2.tile([128, 512], F32R, tag="zsq")
                    act(zsq[:], zt[ot][:].bitcast(F32), AFT.Square)
                    mm(e2ps[:], inv512[:], zsq[:], start=(ot == 0), stop=(ot == CC - 1))
                murow = lp2.tile([1, 512], F32, tag="murow", bufs=1)
                nc.vector.tensor_copy(murow[:], mups[:])
                musq = lp2.tile([1, 512], F32, tag="musq", bufs=1)
                tt(musq[:], murow[:], murow[:], AluOpType.mult)
                varrow = lp2.tile([1, 512], F32, tag="varrow", bufs=1)
                tt(varrow[:], e2ps[:], musq[:], AluOpType.subtract)
                sig = lp2.tile([1, 512], F32, tag="sig", bufs=1)
                act(sig[:], varrow[:], AFT.Sqrt, bias=eps_c[0:1, :])
                rhs2 = lp2.tile([2, 512], F32R, tag="rhs2", bufs=1)
                ts(rhs2[0:1, :], mups[:], -1.0, None, AluOpType.mult)
                sigR = lp2.tile([1, 512], F32R, tag="sigR", bufs=1)
                nc.vector.tensor_copy(sigR[:], sig[:])
                nc.sync.dma_start(rhs2[1:2, :], sigR[:])
                invsF = lp2.tile([1, 512], F32, tag="invsF", bufs=1)
                nc.vector.reciprocal(invsF[:], sig[:])
                invs = lp2.tile([1, 512], F32R, tag="invs", bufs=1)
                nc.vector.tensor_copy(invs[:], invsF[:])
                bc = ps_big.tile([128, 512], F32, tag="big")
                mm(bc[:], ones1x128[:], invs[:], start=True, stop=True)
                invsb = lp2.tile([128, 512], F32, tag="invsb", bufs=1)
                nc.vector.tensor_copy(invsb[:], bc[:])

                # FFN1 + ELU + FFN2 accumulation (j-outer so hE slots rotate)
                f2ps = [ps_big.tile([128, 512], F32, tag="big", name=f"f2ps{o2}")
                        for o2 in range(CC)]
                mu2 = ps_row.tile([1, 512], F32, tag="row", name="mu2")
                for j in range(8):
                    fps = ps_big.tile([128, 512], F32, tag="big", name="fps")
                    for cc in range(CC):
                        mm(fps[:], W1gTt[:, cc, j * 128:(j + 1) * 128], zt[cc][:],
                           start=(cc == 0), stop=False)
                    mm(fps[:], U1W[:, j * 128:(j + 1) * 128], rhs2[:], start=False, stop=True)
                    hp = lp2.tile([128, 512], F32, tag="hp")
                    tt(hp[:], fps[:], invsb[:], AluOpType.mult)
                    E = lp2.tile([128, 512], F32, tag="E")
                    act(E[:], hp[:], AFT.Exp)
                    nc.gpsimd.tensor_scalar(E[:], E[:], 1.0, -1.0, AluOpType.min,
                                            AluOpType.add)
                    he = lph.tile([128, 512], F32R, tag="hE", bufs=3, name="he")
                    stt(he[:], hp[:], 0.0, E[:], AluOpType.max, AluOpType.add)
                    for o2 in range(CC):
                        mm(f2ps[o2][:], W2gTt[:, j, o2 * 128:(o2 + 1) * 128], he[:],
                           start=(j == 0), stop=False, skip_group_check=True)
                    mm(mu2[:], u2ct[:, j:j + 1], he[:], start=(j == 0), stop=(j == 7),
                       skip_group_check=True)
                rhs2b = lp2.tile([2, 512], F32R, tag="rhs2b", bufs=1)
                nc.sync.dma_start(rhs2b[0:1, :], ones_row[:])
                negmu2 = lp2.tile([1, 512], F32R, tag="negmu2", bufs=1)
                ts(negmu2[:], mu2[:], -1.0, B2MEAN_PLACEHOLDER, AluOpType.mult,
                   AluOpType.subtract)
                nc.sync.dma_start(rhs2b[1:2, :], negmu2[:])
                yg = []
                for o2 in range(CC):
                    mm(f2ps[o2][:], G2B[:, o2 * 128:(o2 + 1) * 128], rhs2b[:],
                       start=False, stop=True, skip_group_check=True)
                    y = lph.tile([128, 512], F32, tag=f"yg{o2}", name=f"yg{o2}")
                    nc.vector.tensor_copy(y[:], f2ps[o2][:])
                    yg.append(y)

                # LN2 variance + apply
                v2ps = ps_row.tile([1, 512], F32, tag="row")
                for o2 in range(CC):
                    sq2 = lp2.tile([128, 512], F32R, tag="sq2")
                    act(sq2[:], yg[o2][:], AFT.Square)
                    mm(v2ps[:], ivgt[:, o2:o2 + 1], sq2[:], start=(o2 == 0),
                       stop=(o2 == CC - 1))
                sig2 = lp2.tile([1, 512], F32, tag="sig2", bufs=1)
                act(sig2[:], v2ps[:], AFT.Sqrt, bias=eps_c[0:1, :])
                invs2F = lp2.tile([1, 512], F32, tag="invs2F", bufs=1)
                nc.vector.reciprocal(invs2F[:], sig2[:])
                invs2 = lp2.tile([1, 512], F32R, tag="invs2", bufs=1)
                nc.vector.tensor_copy(invs2[:], invs2F[:])
                bc2 = ps_big.tile([128, 512], F32, tag="big")
                mm(bc2[:], ones1x128[:], invs2[:], start=True, stop=True)
                invsb2 = lp2.tile([128, 512], F32, tag="invsb2", bufs=1)
                nc.vector.tensor_copy(invsb2[:], bc2[:])
                for o2 in range(CC):
                    tt(yg[o2][:], yg[o2][:], invsb2[:], AluOpType.mult)
                    ot_t = lp2.tile([128, 512], F32, tag="ot")
                    nc.gpsimd.tensor_scalar(ot_t[:], yg[o2][:], be2_c[:, o2:o2 + 1],
                                            None, AluOpType.add)
                    nc.sync.dma_start(outd[o2 * 128:(o2 + 1) * 128, sl], ot_t[:])

    nc.compile()
    return nc


def _prep_consts(Wq, bq, Wk, bk, Wv, bv, Wr, br, g1, be1, W1, b1, W2, b2, g2, be2):
    f = np.float32
    WqT = np.ascontiguousarray(Wq.T, dtype=f)                       # [c, o]
    WkvT = np.concatenate([Wk.T, Wv.T], axis=1).astype(f)           # [c, k|v]
    WrT = np.ascontiguousarray(Wr.T, dtype=f)                       # [v, o]
    W1g = (W1 * g1[None, :]).astype(f)                              # [1024, c]
    W1gT = np.ascontiguousarray(W1g.T)                              # [c, 1024]
    W2g = (W2 * g2[:, None]).astype(f)                              # [c, 1024h]
    W2gT = np.ascontiguousarray(W2g.T)                              # [h, c]
    u1 = W1g.sum(axis=1).astype(f)
    w1bb = (W1 @ be1 + b1).astype(f)
    u2 = (W2.sum(axis=0) / 512.0).astype(f)
    ivg = (1.0 / (512.0 * g2 * g2)).astype(f)
    b2mean = float(np.mean(b2))

    def chunkT(a, n):          # [n*128, m] -> [128, n, m]
        return np.ascontiguousarray(a.reshape(n, 128, -1).transpose(1, 0, 2))

    def colsT(v, n):           # [n*128] -> [128, n]
        return np.ascontiguousarray(v.reshape(n, 128).T)

    return {
        "WqTt": chunkT(WqT, CC),
        "bq_row": bq.reshape(1, 512).astype(f),
        "WkvTt": chunkT(WkvT, CC),
        "WrTt": chunkT(WrT, CC),
        "W1gTt": chunkT(W1gT, CC),
        "W2gTt": chunkT(W2gT, 8),
        "U1W": np.stack([u1, w1bb]).astype(f),
        "u2ct": colsT(u2, 8),
        "G2B": np.stack([(g2 * b2).astype(f), g2.astype(f)]),
        "ivgt": colsT(ivg, CC),
        "inv512": np.full((128, 1), 1.0 / 512.0, dtype=f),
        "ones1x128": np.ones((1, 128), dtype=f),
        "ident": np.eye(128, dtype=f).astype(np.dtype("bfloat16") if False else f),
        "br_c": colsT(br.astype(f), CC),
        "bv_c": colsT(bv.astype(f), CC),
        "be2_c": colsT(be2.astype(f), CC),
        "eps_c": np.full((128, 1), EPS, dtype=f),
        "ones_row": np.ones((1, 512), dtype=f),
    }, b2mean


def kernel(**inputs):
    global LAST_RESULT
    import ml_dtypes
    z1 = np.asarray(inputs["z1"], dtype=np.float32)
    z2 = np.asarray(inputs["z2"], dtype=np.float32)
    consts, b2mean = _prep_consts(
        *[np.asarray(inputs[k], dtype=np.float32) for k in
          ["Wq", "bq", "Wk", "bk", "Wv", "bv", "Wr", "br", "g1", "be1",
           "W1", "b1", "W2", "b2", "g2", "be2"]])
    consts["ident"] = np.eye(128, dtype=ml_dtypes.bfloat16)

    key = ("prog", round(b2mean * 1e9))
    if key not in _CACHE:
        global B2MEAN_PLACEHOLDER
        B2MEAN_PLACEHOLDER = b2mean
        _CACHE.clear()
        _CACHE[key] = _build_program()
    nc = _CACHE[key]

    in_maps = []
    for b in range(B):
        m = dict(consts)
        m["z1"] = np.ascontiguousarray(z1[b])
        m["z2"] = np.ascontiguousarray(z2[b])
        in_maps.append(m)

    import os
    trace = bool(int(os.environ.get("KERNEL_TRACE", "0")))
    res = run_bass_kernel_spmd(nc, in_maps, list(range(B)), trace=trace)
    LAST_RESULT = res
    out = np.stack([res.results[b]["out"] for b in range(B)], axis=0)
    return out.astype(np.float32)


B2MEAN_PLACEHOLDER = 0.0



# revision 24
# speedup vs baseline: 2.1731x; 2.1731x over previous
"""CACombiner Trainium2 kernel: conv-projected efficient attention + FFN.

Data-parallel over batch: 8 batch elements -> 8 NeuronCores, identical SPMD
program per core. Attention block (q/k/v, ctx, apply, reprojection) runs in
fp8e4 with DoubleRow matmuls; the FFN runs in float32r. LayerNorm rsqrt is
computed as exp(-0.5*ln(var+eps)) so every activation stays in one ACT table.
"""
import sys
sys.path.insert(0, "/opt/trn_rl_repo")
from contextlib import ExitStack

import numpy as np

import concourse.bass as bass
import concourse.tile as tile
from concourse import mybir, bacc
from concourse.bass_utils import run_bass_kernel_spmd
from concourse.alu_op_type import AluOpType

F32 = mybir.dt.float32
F32R = mybir.dt.float32r
BF16 = mybir.dt.bfloat16
F8 = mybir.dt.float8e4
AFT = mybir.ActivationFunctionType
DR = mybir.MatmulPerfMode.DoubleRow

B, C, L = 8, 512, 4096
H, DK = 8, 64
EPS = 1e-5
CC = C // 128           # 4 channel chunks
NT = L // 512           # 8 l-tiles (512 wide)

_CACHE = {}
LAST_RESULT = None

# compile-time floats baked into the program (set before _build_program)
CONSTS = {}


def _build_program():
    cst = CONSTS
    nc = bacc.Bacc("TRN2", target_bir_lowering=False, debug=False)

    def din(name, shape, dtype):
        return nc.dram_tensor(name, list(shape), dtype, kind="ExternalInput").ap()

    z1q_d = din("z1q", (C, L), F8)
    z2q_d = din("z2q", (C, L), F8)
    z1rb_d = din("z1rb", (C, L), BF16)
    wq8_d = din("wq8", (128, CC, 512), F8)
    wkv8_d = din("wkv8", (128, CC, 1024), F8)
    wr8_d = din("wr8", (128, CC, 512), F8)
    w1g_d = din("w1g", (128, CC, 1024), F32R)
    u1wa_d = din("u1wa", (1, 1024), F32R)
    u1wb_d = din("u1wb", (1, 1024), F32R)
    w2c_d = din("w2c", (128, 8, 512), F32R)
    u2c_d = din("u2c", (128, 8), F32R)
    g3g_d = din("g3g", (1, 512), F32R)
    g3be_d = din("g3be", (1, 512), F32R)
    ones512_d = din("ones512", (1, 512), F32R)
    selq_d = din("selq", (128, CC, 8), F8)
    selbc_d = din("selbc", (8, CC, 128), F32R)
    eqb_d = din("eqb", (128, CC), F32)
    ivg2_d = din("ivg2", (128, CC), F32)
    ybc_d = din("ybc", (128, CC), F32)
    gyb_d = din("gyb", (128, CC), F32)
    inv512_d = din("inv512", (128, 1), F32R)
    inv512b_d = din("inv512b", (128, 1), BF16)
    ones1x128_d = din("ones1x128", (1, 128), F32R)
    misc_d = din("misc", (128, 2), F32)
    outd = nc.dram_tensor("out", [C, L], F32, kind="ExternalOutput").ap()

    z1qr = z1q_d.rearrange("(cc p) l -> p cc l", p=128)
    z2qr = z2q_d.rearrange("(cc p) l -> p cc l", p=128)
    z1rbr = z1rb_d.rearrange("(cc p) l -> p cc l", p=128)
    outr = outd.rearrange("(cc p) l -> p cc l", p=128)

    mm = nc.tensor.matmul
    tt = nc.vector.tensor_tensor
    ts = nc.vector.tensor_scalar
    stt = nc.vector.scalar_tensor_tensor
    act = nc.scalar.activation

    with tile.TileContext(nc) as tc, ExitStack() as ctx:
        cpool = ctx.enter_context(tc.tile_pool(name="consts", bufs=1))

        def const_tile(shape, dtype, src, tag):
            t = cpool.tile(list(shape), dtype, tag=tag, name=tag)
            nc.sync.dma_start(t[:], src)
            return t

        # phase-1 weights + small consts first (so phase 1 starts fast)
        wq8 = const_tile((128, CC, 512), F8, wq8_d, "wq8")
        wkv8 = const_tile((128, CC, 1024), F8, wkv8_d, "wkv8")
        wr8 = const_tile((128, CC, 512), F8, wr8_d, "wr8")
        selq = const_tile((128, CC, 8), F8, selq_d, "selq")
        selbc = const_tile((8, CC, 128), F32R, selbc_d, "selbc")
        eqb = const_tile((128, CC), F32, eqb_d, "eqb")
        ivg2 = const_tile((128, CC), F32, ivg2_d, "ivg2")
        ybc = const_tile((128, CC), F32, ybc_d, "ybc")
        gyb = const_tile((128, CC), F32, gyb_d, "gyb")
        inv512 = const_tile((128, 1), F32R, inv512_d, "inv512")
        inv512b = const_tile((128, 1), BF16, inv512b_d, "inv512b")
        ones1x128 = const_tile((1, 128), F32R, ones1x128_d, "ones1x128")
        u1wa = const_tile((1, 1024), F32R, u1wa_d, "u1wa")
        if not cst["w1bb0"] > 0.5:
            u1wb = const_tile((1, 1024), F32R, u1wb_d, "u1wb")
            ones512 = const_tile((1, 512), F32R, ones512_d, "ones512")
        u2c = const_tile((128, 8), F32R, u2c_d, "u2c")
        g3g = const_tile((1, 512), F32R, g3g_d, "g3g")
        g3be = const_tile((1, 512), F32R, g3be_d, "g3be")
        misc = const_tile((128, 2), F32, misc_d, "misc")

        # big FFN weights: DMA issued inside the phase-1 loop (overlap)
        w1g = cpool.tile([128, CC, 1024], F32R, tag="w1g", name="w1g")
        w2c = cpool.tile([128, 8, 512], F32R, tag="w2c", name="w2c")

        # persistent across phases
        qsm8 = cpool.tile([128, CC, L], F8, tag="qsm8", name="qsm8")
        ctxbd = [cpool.tile([128, 128], F8, tag=f"ctxbd{p}", name=f"ctxbd{p}")
                 for p in range(CC)]
        nc.scalar.add_instruction(mybir.InstLoadActFuncSet(
            name=nc.get_next_instruction_name(), ins=[], outs=[],
            act_func_set_id=6))

        # ---------------- Phase 1: q/k/v + exp + ctx accumulation ----------------
        with ExitStack() as p1:
            lp1 = p1.enter_context(tc.tile_pool(name="lp1", bufs=2))
            ps_kv = p1.enter_context(tc.tile_pool(name="ps_kv", bufs=2, space="PSUM"))
            ps_q = p1.enter_context(tc.tile_pool(name="ps_q", bufs=2, space="PSUM"))
            ps_ctx = p1.enter_context(tc.tile_pool(name="ps_ctx", bufs=1, space="PSUM"))

            ctxps = [ps_ctx.tile([128, 2, 129], F32, tag=f"ctx{i}", name=f"ctxps{i}")
                     for i in range(2)]

            pend = None       # lagged ctx MM emission: (ek8, v8t, is_first)
            first_ctx = True

            def flush_ctx(last=False):
                nonlocal pend, first_ctx
                if pend is None:
                    return
                ek8_, v8t_, _ = pend
                for s01 in range(2):
                    for pr in range(CC):
                        mm(ctxps[pr // 2][:, pr % 2, :],
                           ek8_[:, s01, pr * 128:(pr + 1) * 128],
                           v8t_[:, s01, pr, :],
                           start=(first_ctx and s01 == 0),
                           stop=(last and s01 == 1),
                           skip_group_check=True)
                first_ctx = False
                pend = None

            for t in range(NT):
                sl = slice(t * 512, (t + 1) * 512)
                z1t = lp1.tile([128, CC, 512], F8, tag="z1t")
                nc.sync.dma_start(z1t[:], z1qr[:, :, sl])
                z2t = lp1.tile([128, CC, 512], F8, tag="z2t")
                nc.sync.dma_start(z2t[:], z2qr[:, :, sl])
                # stream the big FFN weights during phase 1
                if t == 1:
                    nc.sync.dma_start(w1g[:], w1g_d)
                elif t == 2:
                    nc.sync.dma_start(w2c[:], w2c_d)

                # ---- q in [o, l]: per-oc psum, rotation 2 ----
                for oc in range(CC):
                    qps = ps_q.tile([128, 512], F32, tag="qps", name="qps")
                    for p in range(2):
                        mm(qps[:],
                           wq8[:, 2 * p:2 * p + 2, oc * 128:(oc + 1) * 128],
                           z1t[:, 2 * p:2 * p + 2, :],
                           start=(p == 0), stop=(p == 1), perf_mode=DR)
                    act(qsm8[:, oc, sl], qps[:], AFT.Exp,
                        bias=eqb[:, oc:oc + 1], scale=cst["c_q"])

                # ---- k/v per 128-l subtile; ctx lagged one sub-pair ----
                for sp in range(2):
                    ek8 = lp1.tile([128, 2, 512], F8, tag="ek8")
                    v8t = lp1.tile([128, 2, CC, 129], F8, tag="v8t")
                    nc.vector.memset(v8t[:, :, :, 128:129], 1.0)
                    for s01 in range(2):
                        sub = sp * 2 + s01
                        ssl = slice(sub * 128, (sub + 1) * 128)
                        kps = ps_kv.tile([128, 512], F32, tag="kps", name="kps")
                        vps = ps_kv.tile([128, 512], F32, tag="vps", name="vps")
                        for p in range(2):
                            mm(kps[:],
                               z2t[:, 2 * p:2 * p + 2, ssl],
                               wkv8[:, 2 * p:2 * p + 2, 0:512],
                               start=(p == 0), stop=(p == 1), perf_mode=DR)
                        act(ek8[:, s01, :], kps[:], AFT.Exp,
                            bias=misc[:, 0:1], scale=cst["c_k"])
                        for p in range(2):
                            mm(vps[:],
                               z2t[:, 2 * p:2 * p + 2, ssl],
                               wkv8[:, 2 * p:2 * p + 2, 512:1024],
                               start=(p == 0), stop=(p == 1), perf_mode=DR)
                        ts(v8t[:, s01, :, 0:128],
                           vps[:].rearrange("p (cc x) -> p cc x", x=128),
                           cst["c_v"], None, AluOpType.mult)
                    flush_ctx()
                    pend = (ek8, v8t, None)
            flush_ctx(last=True)

            # ---- finalize ctx: normalize rows, build block-diagonal fp8 ----
            rsc = lp1.tile([128, 2, 2], F32, tag="rsc", bufs=1)
            for i in range(2):
                nc.vector.reciprocal(rsc[:, i, :], ctxps[i][:, :, 128])
            for pr in range(CC):
                nc.vector.memset(ctxbd[pr][:], 0.0)
                for hb in range(2):
                    hs = slice(hb * 64, (hb + 1) * 64)
                    ts(ctxbd[pr][hs, hs],
                       ctxps[pr // 2][hs, pr % 2, hs],
                       rsc[hs, pr // 2, pr % 2:pr % 2 + 1],
                       cst["s_cv"], AluOpType.mult, AluOpType.mult)

        # ---------------- Phase 2: apply + reprojection + LN/FFN ----------------
        with ExitStack() as p2:
            lp2 = p2.enter_context(tc.tile_pool(name="lp2", bufs=2))
            lph = p2.enter_context(tc.tile_pool(name="lph", bufs=2))
            ps_t = p2.enter_context(tc.tile_pool(name="ps_t", bufs=2, space="PSUM"))
            ps_fp = p2.enter_context(tc.tile_pool(name="ps_fp", bufs=2, space="PSUM"))
            ps_f2 = p2.enter_context(tc.tile_pool(name="ps_f2", bufs=2, space="PSUM"))
            ps_row = p2.enter_context(tc.tile_pool(name="ps_row", bufs=1, space="PSUM"))

            def stage_A(t):
                sl = slice(t * 512, (t + 1) * 512)
                z1res = lp2.tile([128, CC, 512], BF16, tag="z1res", name="z1res")
                nc.sync.dma_start(z1res[:], z1rbr[:, :, sl])
                sqp = ps_row.tile([8, 512], F32, tag="aux", name="sqp")

                # q softmax denominators
                for pr in range(CC):
                    mm(sqp[:], selq[:, pr, :], qsm8[:, pr, sl],
                       start=(pr == 0), stop=(pr == CC - 1), skip_group_check=True)
                rq = lp2.tile([8, 512], F32R, tag="rq", name="rq")
                with nc.allow_low_precision(reason="rq rounds to f32r for bcast matmul"):
                    nc.vector.reciprocal(rq[:], sqp[:])

                # attention apply
                att8 = lp2.tile([128, CC, 512], F8, tag="att8", name="att8")
                rqbs = lp2.tile([128, CC, 512], BF16, tag="rqbs", bufs=1,
                                name="rqbs")
                for pr in range(CC):
                    rqb = ps_t.tile([128, 512], F32, tag="pst", name="rqb")
                    mm(rqb[:], selbc[:, pr, :], rq[:],
                       start=True, stop=True)
                    act(rqbs[:, pr, :], rqb[:], AFT.Copy)
                    attps = ps_t.tile([128, 512], F32, tag="pst", name="attps")
                    mm(attps[:], ctxbd[pr][:], qsm8[:, pr, sl],
                       start=True, stop=True)
                    tt(att8[:, pr, :], attps[:], rqbs[:, pr, :], AluOpType.mult)

                # z = Wr att + z1 + biases
                zt = lp2.tile([128, CC, 512], F32R, tag="zt", bufs=1, name="zt")
                zsq = lp2.tile([128, CC, 512], BF16, tag="zsq", bufs=1, name="zsq")
                for oc in range(CC):
                    zps = ps_t.tile([128, 512], F32, tag="pst", name="zps")
                    for p in range(2):
                        mm(zps[:], wr8[:, 2 * p:2 * p + 2, oc * 128:(oc + 1) * 128],
                           att8[:, 2 * p:2 * p + 2, :],
                           start=(p == 0), stop=(p == 1), perf_mode=DR)
                    stt(zt[:, oc, :], zps[:], cst["c_z"],
                        z1res[:, oc, :], AluOpType.mult, AluOpType.add)
                    act(zsq[:, oc, :], zt[:, oc, :].bitcast(F32), AFT.Square)
                # LN1 stats: mu then e2 through one rotating psum bank
                mu1p = ps_row.tile([1, 512], F32, tag="rowacc", name="mu1p")
                for oc in range(CC):
                    mm(mu1p[:], inv512[:], zt[:, oc, :],
                       start=(oc == 0), stop=(oc == CC - 1), skip_group_check=True)
                mu1row = lp2.tile([1, 512], F32, tag="mu1row", name="mu1row")
                act(mu1row[:], mu1p[:], AFT.Copy)
                e2p = ps_row.tile([1, 512], F32, tag="rowacc", name="e2p")
                for oc in range(CC):
                    mm(e2p[:], inv512b[:], zsq[:, oc, :],
                       start=(oc == 0), stop=(oc == CC - 1), skip_group_check=True)

                # LN1 rows: rsig1 = exp(-0.5 ln(var+eps))
                musq1 = lp2.tile([1, 512], F32, tag="musq1", name="musq1")
                act(musq1[:], mu1row[:], AFT.Square)
                var1 = lp2.tile([1, 512], F32, tag="var1", name="var1")
                tt(var1[:], e2p[:], musq1[:], AluOpType.subtract)
                lnv1 = lp2.tile([1, 512], F32, tag="lnv1", name="lnv1")
                act(lnv1[:], var1[:], AFT.Ln, bias=misc[0:1, 1:2])
                rs1 = lp2.tile([1, 512], F32R, tag="rs1", name="rs1")
                act(rs1[:], lnv1[:], AFT.Exp, scale=-0.5)
                mneg = lp2.tile([1, 512], F32R, tag="mneg", name="mneg")
                tt(mneg[:], mu1row[:], rs1[:], AluOpType.mult)
                rs1b = ps_row.tile([128, 512], F32, tag="aux", name="rs1b")
                mm(rs1b[:], ones1x128[:], rs1[:], start=True, stop=True,
                   skip_group_check=True)

                # zn = z * rsig1
                zn = lp2.tile([128, CC, 512], F32R, tag="zn", name="zn")
                for oc in range(CC):
                    tt(zn[:, oc, :], zt[:, oc, :].bitcast(F32), rs1b[:],
                       AluOpType.mult)
                return sl, zn, mneg

            def stage_B(st):
                sl, zn, mneg = st

                # FFN1 + ELU
                he2 = lph.tile([128, 8, 512], F32R, tag="he2", bufs=1, name="he2")
                for j in range(8):
                    js = slice(j * 128, (j + 1) * 128)
                    fps = ps_fp.tile([128, 512], F32, tag="fps", name="fps")
                    for cc2 in range(CC):
                        mm(fps[:], w1g[:, cc2, js], zn[:, cc2, :],
                           start=(cc2 == 0), stop=False)
                    mm(fps[:], u1wa[:, js], mneg[:], start=False,
                       stop=cst["w1bb0"] > 0.5, skip_group_check=True)
                    if not cst["w1bb0"] > 0.5:
                        mm(fps[:], u1wb[:, js], ones512[:], start=False,
                           stop=True, skip_group_check=True)
                    E = lp2.tile([128, 512], BF16, tag="E", bufs=2, name="E")
                    act(E[:], fps[:], AFT.Exp)
                    t2 = lp2.tile([128, 512], BF16, tag="t2", bufs=2, name="t2")
                    nc.gpsimd.tensor_scalar(t2[:], E[:], 1.0, None, AluOpType.min)
                    stt(he2[:, j, :], fps[:], 0.0, t2[:], AluOpType.max,
                        AluOpType.add)
                # LN2 mean rows (u2c . he2)
                mu2p = ps_row.tile([1, 512], F32, tag="rowacc", name="mu2p")
                for j in range(8):
                    mm(mu2p[:], u2c[:, j:j + 1], he2[:, j, :],
                       start=(j == 0), stop=(j == 7), skip_group_check=True)
                mu2row = lp2.tile([1, 512], F32, tag="mu2row", name="mu2row")
                act(mu2row[:], mu2p[:], AFT.Copy)
                e2yp = ps_row.tile([1, 512], F32, tag="rowacc", name="e2yp")

                # FFN2 o-outer + LN2 stats
                g2y = lph.tile([128, CC, 512], F32R, tag="g2y", bufs=1, name="g2y")
                for o2 in range(CC):
                    os_ = slice(o2 * 128, (o2 + 1) * 128)
                    f2ps = ps_f2.tile([128, 512], F32, tag="f2", name="f2ps")
                    for j in range(8):
                        mm(f2ps[:], w2c[:, j, os_], he2[:, j, :],
                           start=(j == 0), stop=(j == 7))
                    ysq = lp2.tile([128, 512], F32R, tag="ysq", name="ysq")
                    act(ysq[:], f2ps[:], AFT.Square,
                        bias=ybc[:, o2:o2 + 1], scale=ivg2[:, o2:o2 + 1])
                    mm(e2yp[:], inv512[:], ysq[:],
                       start=(o2 == 0), stop=(o2 == CC - 1), skip_group_check=True)
                    act(g2y[:, o2, :], f2ps[:], AFT.Identity,
                        bias=gyb[:, o2:o2 + 1])

                # LN2 rows
                negmu2 = lp2.tile([1, 512], F32R, tag="negmu2", name="negmu2")
                ts(negmu2[:], mu2row[:], cst["kc"], -1.0,
                   AluOpType.add, AluOpType.mult)
                musq2 = lp2.tile([1, 512], F32, tag="musq2", name="musq2")
                tt(musq2[:], negmu2[:].bitcast(F32), negmu2[:].bitcast(F32),
                   AluOpType.mult)
                var2 = lp2.tile([1, 512], F32, tag="var2", name="var2")
                tt(var2[:], e2yp[:], musq2[:], AluOpType.subtract)
                lnv2 = lp2.tile([1, 512], F32, tag="lnv2", name="lnv2")
                act(lnv2[:], var2[:], AFT.Ln, bias=misc[0:1, 1:2])
                rs2 = lp2.tile([1, 512], F32R, tag="rs2", name="rs2")
                act(rs2[:], lnv2[:], AFT.Exp, scale=-0.5)
                sig2 = lp2.tile([1, 512], F32R, tag="sig2", name="sig2")
                act(sig2[:], lnv2[:], AFT.Exp, scale=0.5)
                rs2b = ps_row.tile([128, 512], F32, tag="aux", name="rs2b")
                mm(rs2b[:], ones1x128[:], rs2[:], start=True, stop=True,
                   skip_group_check=True)

                # finalize: out = (g2y + g2(x)negmu2 + be2(x)sig2) * rsig2
                outt = lp2.tile([128, CC, 512], F32, tag="outt", bufs=1, name="outt")
                for o2 in range(CC):
                    os_ = slice(o2 * 128, (o2 + 1) * 128)
                    Rp = ps_t.tile([128, 512], F32, tag="pst", name="Rp")
                    mm(Rp[:], g3g[:, os_], negmu2[:], start=True, stop=False)
                    mm(Rp[:], g3be[:, os_], sig2[:], start=False, stop=True)
                    w_ = lp2.tile([128, 512], F32, tag="w_", name="w_")
                    tt(w_[:], g2y[:, o2, :].bitcast(F32), Rp[:], AluOpType.add)
                    tt(outt[:, o2, :], w_[:], rs2b[:], AluOpType.mult)
                nc.sync.dma_start(outr[:, :, sl], outt[:])

            stA = stage_A(0)
            for t in range(1, NT):
                stA_next = stage_A(t)
                stage_B(stA)
                stA = stA_next
            stage_B(stA)

    nc.compile()
    return nc


def _prep_consts(z1, z2, Wq, bq, Wk, bk, Wv, bv, Wr, br, g1, be1,
                 W1, b1, W2, b2, g2, be2):
    import ml_dtypes
    f = np.float32
    F8NP = ml_dtypes.float8_e4m3

    def wscale(W):
        m = float(np.abs(W).max())
        return 160.0 / m if m > 0 else 1.0

    s1 = 160.0 / max(float(np.abs(z1).max()), 1e-30)
    s2 = 160.0 / max(float(np.abs(z2).max()), 1e-30)
    swq, swk, swv, swr = wscale(Wq), wscale(Wk), wscale(Wv), wscale(Wr)
    s_q, s_e, s_v, s_c, s_a = 4.0, 4.0, 32.0, 32.0, 32.0

    def chunkT(a, s):          # [out, in] -> [128, in//128, out] scaled fp8
        o, cin = a.shape
        return np.ascontiguousarray(
            (a.T * s).reshape(cin // 128, 128, o).transpose(1, 0, 2)).astype(F8NP)

    wq8 = chunkT(Wq, swq)                                       # [128, 4, 512]
    wkv = np.concatenate([Wk.T * swk, Wv.T * swv], axis=1)      # [512, 1024]
    wkv8 = np.ascontiguousarray(
        wkv.reshape(CC, 128, 1024).transpose(1, 0, 2)).astype(F8NP)
    wr8 = chunkT(Wr, swr)                                       # [128, 4, 512]

    W1g = (W1 * g1[None, :]).astype(f)                          # [1024, 512]
    w1g = np.ascontiguousarray(
        W1g.T.reshape(CC, 128, 1024).transpose(1, 0, 2)).astype(f)
    u1 = W1g.sum(axis=1).astype(f)
    w1bb = (W1 @ be1 + b1).astype(f)
    u1wa = (-u1).reshape(1, 1024).astype(f)
    u1wb = w1bb.reshape(1, 1024).astype(f)

    W2g = (W2 * g2[:, None]).astype(f)                          # [512, 1024]
    w2c = np.ascontiguousarray(
        W2g.T.reshape(8, 128, 512).transpose(1, 0, 2)).astype(f)
    u2 = (W2.sum(axis=0) / 512.0).astype(f)                     # [1024]
    u2c = np.ascontiguousarray(u2.reshape(8, 128).T).astype(f)  # [128, 8]
    w2sum = W2.sum(axis=1).astype(f)                            # [512]
    kc = float(np.mean(b2) - np.mean(w2sum))
    g3g = g2.reshape(1, 512).astype(f)
    g3be = be2.reshape(1, 512).astype(f)

    def colsC(v):
        return np.ascontiguousarray(np.asarray(v, f).reshape(CC, 128).T)

    selq = np.zeros((128, CC, 8), dtype=F8NP)
    selbc = np.zeros((8, CC, 128), dtype=f)
    for pr in range(CC):
        for hb in range(2):
            h = 2 * pr + hb
            selq[hb * 64:(hb + 1) * 64, pr, h] = 1.0
            selbc[h, pr, hb * 64:(hb + 1) * 64] = s_a / s_c

    consts = {
        "wq8": wq8, "wkv8": wkv8, "wr8": wr8,
        "w1g": w1g, "u1wa": u1wa, "u1wb": u1wb, "w2c": w2c, "u2c": u2c,
        "g3g": g3g, "g3be": g3be,
        "ones512": np.ones((1, 512), dtype=f),
        "selq": selq, "selbc": selbc,
        "eqb": colsC(bq + np.log(s_q)),
        "ivg2": colsC(1.0 / g2),
        "ybc": colsC(b2 - w2sum),
        "gyb": colsC(g2 * (b2 - w2sum)),
        "inv512": np.full((128, 1), 1.0 / 512.0, dtype=f),
        "inv512b": np.full((128, 1), 1.0 / 512.0, dtype=np.dtype("bfloat16") if False else f).astype(__import__("ml_dtypes").bfloat16),
        "ones1x128": np.ones((1, 128), dtype=f),
        "misc": np.stack([np.full(128, np.log(s_e), dtype=f), np.full(128, EPS, dtype=f)], axis=1),
    }
    scalars = {
        "c_q": 1.0 / (swq * s1),
        "c_k": 1.0 / (swk * s2),
        "ln_se": float(np.log(s_e)),
        "c_v": s_v / (swv * s2),
        "s_cv": s_c / s_v,
        "c_z": 1.0 / (swr * s_a),
        "kc": kc,
        "w1bb0": 1.0 if not np.any(w1bb) else 0.0,
    }
    import ml_dtypes as mld
    z1rb = (z1 + (br + Wr @ bv)[None, :, None]).astype(mld.bfloat16)
    return consts, scalars, z1rb, s1, s2


def kernel(**inputs):
    global LAST_RESULT, CONSTS
    import ml_dtypes
    F8NP = ml_dtypes.float8_e4m3
    z1 = np.asarray(inputs["z1"], dtype=np.float32)
    z2 = np.asarray(inputs["z2"], dtype=np.float32)
    wargs = [np.asarray(inputs[k], dtype=np.float32) for k in
             ["Wq", "bq", "Wk", "bk", "Wv", "bv", "Wr", "br", "g1", "be1",
              "W1", "b1", "W2", "b2", "g2", "be2"]]
    consts, scalars, z1rb, s1, s2 = _prep_consts(z1, z2, *wargs)

    key = ("prog",) + tuple(sorted((k, round(v * 1e9)) for k, v in scalars.items()))
    if key not in _CACHE:
        CONSTS = scalars
        _CACHE.clear()
        _CACHE[key] = _build_program()
    nc = _CACHE[key]

    z1q = np.ascontiguousarray((z1 * s1).astype(F8NP))
    z2q = np.ascontiguousarray((z2 * s2).astype(F8NP))

    in_maps = []
    for b in range(B):
        m = dict(consts)
        m["z1q"] = z1q[b]
        m["z2q"] = z2q[b]
        m["z1rb"] = np.ascontiguousarray(z1rb[b])
        in_maps.append(m)

    import os
    trace = bool(int(os.environ.get("KERNEL_TRACE", "0")))
    res = run_bass_kernel_spmd(nc, in_maps, list(range(B)), trace=trace)
    LAST_RESULT = res
    out = np.stack([res.results[b]["out"] for b in range(B)], axis=0)
    return out.astype(np.float32)


# revision 26
# speedup vs baseline: 2.2149x; 1.0193x over previous
"""CACombiner Trainium2 kernel: conv-projected efficient attention + FFN.

Data-parallel over batch: 8 batch elements -> 8 NeuronCores, identical SPMD
program per core. Attention block (q/k/v, ctx, apply, reprojection) runs in
fp8e4 with DoubleRow matmuls; the FFN runs in float32r. LayerNorm rsqrt is
computed as exp(-0.5*ln(var+eps)) so every activation stays in one ACT table.
"""
import sys
sys.path.insert(0, "/opt/trn_rl_repo")
from contextlib import ExitStack

import numpy as np

import concourse.bass as bass
import concourse.tile as tile
from concourse import mybir, bacc
from concourse.bass_utils import run_bass_kernel_spmd
from concourse.alu_op_type import AluOpType

F32 = mybir.dt.float32
F32R = mybir.dt.float32r
BF16 = mybir.dt.bfloat16
F8 = mybir.dt.float8e4
AFT = mybir.ActivationFunctionType
DR = mybir.MatmulPerfMode.DoubleRow

B, C, L = 8, 512, 4096
H, DK = 8, 64
EPS = 1e-5
CC = C // 128           # 4 channel chunks
NT = L // 512           # 8 l-tiles (512 wide)

_CACHE = {}
LAST_RESULT = None

# compile-time floats baked into the program (set before _build_program)
CONSTS = {}


def _build_program():
    cst = CONSTS
    nc = bacc.Bacc("TRN2", target_bir_lowering=False, debug=False)

    def din(name, shape, dtype):
        return nc.dram_tensor(name, list(shape), dtype, kind="ExternalInput").ap()

    z1q_d = din("z1q", (C, L), F8)
    z2q_d = din("z2q", (C, L), F8)
    z1rb_d = din("z1rb", (C, L), BF16)
    wq8_d = din("wq8", (128, CC, 512), F8)
    wkv8_d = din("wkv8", (128, CC, 1024), F8)
    wr8_d = din("wr8", (128, CC, 512), F8)
    w1g_d = din("w1g", (128, CC, 1024), F32R)
    u1wa_d = din("u1wa", (1, 1024), F32R)
    u1wb_d = din("u1wb", (1, 1024), F32R)
    w2c_d = din("w2c", (128, 8, 512), F32R)
    u2c_d = din("u2c", (128, 8), F32R)
    g3g_d = din("g3g", (1, 512), F32R)
    g3be_d = din("g3be", (1, 512), F32R)
    ones512_d = din("ones512", (1, 512), F32R)
    selq_d = din("selq", (128, CC, 8), F8)
    selbc_d = din("selbc", (8, CC, 128), F32R)
    eqb_d = din("eqb", (128, CC), F32)
    ivg2_d = din("ivg2", (128, CC), F32)
    ybc_d = din("ybc", (128, CC), F32)
    gyb_d = din("gyb", (128, CC), F32)
    be2c_d = din("be2c", (128, CC), F32)
    inv512_d = din("inv512", (128, 1), F32R)
    inv512b_d = din("inv512b", (128, 1), BF16)
    ones1x128_d = din("ones1x128", (1, 128), F32R)
    misc_d = din("misc", (128, 2), F32)
    outd = nc.dram_tensor("out", [C, L], F32, kind="ExternalOutput").ap()

    z1qr = z1q_d.rearrange("(cc p) l -> p cc l", p=128)
    z2qr = z2q_d.rearrange("(cc p) l -> p cc l", p=128)
    z1rbr = z1rb_d.rearrange("(cc p) l -> p cc l", p=128)
    outr = outd.rearrange("(cc p) l -> p cc l", p=128)

    mm = nc.tensor.matmul
    tt = nc.vector.tensor_tensor
    ts = nc.vector.tensor_scalar
    stt = nc.vector.scalar_tensor_tensor
    act = nc.scalar.activation

    with tile.TileContext(nc) as tc, ExitStack() as ctx:
        cpool = ctx.enter_context(tc.tile_pool(name="consts", bufs=1))

        def const_tile(shape, dtype, src, tag):
            t = cpool.tile(list(shape), dtype, tag=tag, name=tag)
            nc.sync.dma_start(t[:], src)
            return t

        # phase-1 weights + small consts first (so phase 1 starts fast)
        wq8 = const_tile((128, CC, 512), F8, wq8_d, "wq8")
        wkv8 = const_tile((128, CC, 1024), F8, wkv8_d, "wkv8")
        wr8 = const_tile((128, CC, 512), F8, wr8_d, "wr8")
        selq = const_tile((128, CC, 8), F8, selq_d, "selq")
        selbc = const_tile((8, CC, 128), F32R, selbc_d, "selbc")
        eqb = const_tile((128, CC), F32, eqb_d, "eqb")
        ivg2 = const_tile((128, CC), F32, ivg2_d, "ivg2")
        ybc = const_tile((128, CC), F32, ybc_d, "ybc")
        gyb = const_tile((128, CC), F32, gyb_d, "gyb")
        be2c = const_tile((128, CC), F32, be2c_d, "be2c")
        inv512 = const_tile((128, 1), F32R, inv512_d, "inv512")
        inv512b = const_tile((128, 1), BF16, inv512b_d, "inv512b")
        ones1x128 = const_tile((1, 128), F32R, ones1x128_d, "ones1x128")
        u1wa = const_tile((1, 1024), F32R, u1wa_d, "u1wa")
        if not cst["w1bb0"] > 0.5:
            u1wb = const_tile((1, 1024), F32R, u1wb_d, "u1wb")
            ones512 = const_tile((1, 512), F32R, ones512_d, "ones512")
        u2c = const_tile((128, 8), F32R, u2c_d, "u2c")
        g3g = const_tile((1, 512), F32R, g3g_d, "g3g")
        g3be = const_tile((1, 512), F32R, g3be_d, "g3be")
        misc = const_tile((128, 2), F32, misc_d, "misc")

        # big FFN weights: DMA issued inside the phase-1 loop (overlap)
        w1g = cpool.tile([128, CC, 1024], F32R, tag="w1g", name="w1g")
        w2c = cpool.tile([128, 8, 512], F32R, tag="w2c", name="w2c")

        # persistent across phases
        qsm8 = cpool.tile([128, CC, L], F8, tag="qsm8", name="qsm8")
        ctxbd = [cpool.tile([128, 128], F8, tag=f"ctxbd{p}", name=f"ctxbd{p}")
                 for p in range(CC)]
        nc.scalar.add_instruction(mybir.InstLoadActFuncSet(
            name=nc.get_next_instruction_name(), ins=[], outs=[],
            act_func_set_id=6))

        # ---------------- Phase 1: q/k/v + exp + ctx accumulation ----------------
        with ExitStack() as p1:
            lp1 = p1.enter_context(tc.tile_pool(name="lp1", bufs=2))
            ps_kv = p1.enter_context(tc.tile_pool(name="ps_kv", bufs=2, space="PSUM"))
            ps_q = p1.enter_context(tc.tile_pool(name="ps_q", bufs=2, space="PSUM"))
            ps_ctx = p1.enter_context(tc.tile_pool(name="ps_ctx", bufs=1, space="PSUM"))

            ctxps = [ps_ctx.tile([128, 2, 129], F32, tag=f"ctx{i}", name=f"ctxps{i}")
                     for i in range(2)]

            pend = None       # lagged ctx MM emission: (ek8, v8t, is_first)
            first_ctx = True

            def flush_ctx(last=False):
                nonlocal pend, first_ctx
                if pend is None:
                    return
                ek8_, v8t_, _ = pend
                for s01 in range(2):
                    for pr in range(CC):
                        mm(ctxps[pr // 2][:, pr % 2, :],
                           ek8_[:, s01, pr * 128:(pr + 1) * 128],
                           v8t_[:, s01, pr, :],
                           start=(first_ctx and s01 == 0),
                           stop=(last and s01 == 1),
                           skip_group_check=True)
                first_ctx = False
                pend = None

            for t in range(NT):
                sl = slice(t * 512, (t + 1) * 512)
                z1t = lp1.tile([128, CC, 512], F8, tag="z1t")
                nc.sync.dma_start(z1t[:], z1qr[:, :, sl])
                z2t = lp1.tile([128, CC, 512], F8, tag="z2t")
                nc.sync.dma_start(z2t[:], z2qr[:, :, sl])
                # stream the big FFN weights during phase 1
                if t == 1:
                    nc.sync.dma_start(w1g[:], w1g_d)
                elif t == 2:
                    nc.sync.dma_start(w2c[:], w2c_d)

                # ---- q in [o, l]: per-oc psum, rotation 2 ----
                for oc in range(CC):
                    qps = ps_q.tile([128, 512], F32, tag="qps", name="qps")
                    for p in range(2):
                        mm(qps[:],
                           wq8[:, 2 * p:2 * p + 2, oc * 128:(oc + 1) * 128],
                           z1t[:, 2 * p:2 * p + 2, :],
                           start=(p == 0), stop=(p == 1), perf_mode=DR)
                    act(qsm8[:, oc, sl], qps[:], AFT.Exp,
                        bias=eqb[:, oc:oc + 1], scale=cst["c_q"])

                # ---- k/v per 128-l subtile; ctx lagged one sub-pair ----
                for sp in range(2):
                    ek8 = lp1.tile([128, 2, 512], F8, tag="ek8")
                    v8t = lp1.tile([128, 2, CC, 129], F8, tag="v8t")
                    nc.vector.memset(v8t[:, :, :, 128:129], 1.0)
                    for s01 in range(2):
                        sub = sp * 2 + s01
                        ssl = slice(sub * 128, (sub + 1) * 128)
                        kps = ps_kv.tile([128, 512], F32, tag="kps", name="kps")
                        vps = ps_kv.tile([128, 512], F32, tag="vps", name="vps")
                        for p in range(2):
                            mm(kps[:],
                               z2t[:, 2 * p:2 * p + 2, ssl],
                               wkv8[:, 2 * p:2 * p + 2, 0:512],
                               start=(p == 0), stop=(p == 1), perf_mode=DR)
                        act(ek8[:, s01, :], kps[:], AFT.Exp,
                            bias=misc[:, 0:1], scale=cst["c_k"])
                        for p in range(2):
                            mm(vps[:],
                               z2t[:, 2 * p:2 * p + 2, ssl],
                               wkv8[:, 2 * p:2 * p + 2, 512:1024],
                               start=(p == 0), stop=(p == 1), perf_mode=DR)
                        ts(v8t[:, s01, :, 0:128],
                           vps[:].rearrange("p (cc x) -> p cc x", x=128),
                           cst["c_v"], None, AluOpType.mult)
                    flush_ctx()
                    pend = (ek8, v8t, None)
            flush_ctx(last=True)

            # ---- finalize ctx: normalize rows, build block-diagonal fp8 ----
            rsc = lp1.tile([128, 2, 2], F32, tag="rsc", bufs=1)
            for i in range(2):
                nc.vector.reciprocal(rsc[:, i, :], ctxps[i][:, :, 128])
            for pr in range(CC):
                nc.vector.memset(ctxbd[pr][:], 0.0)
                for hb in range(2):
                    hs = slice(hb * 64, (hb + 1) * 64)
                    ts(ctxbd[pr][hs, hs],
                       ctxps[pr // 2][hs, pr % 2, hs],
                       rsc[hs, pr // 2, pr % 2:pr % 2 + 1],
                       cst["s_cv"], AluOpType.mult, AluOpType.mult)

        # ---------------- Phase 2: apply + reprojection + LN/FFN ----------------
        with ExitStack() as p2:
            lp2 = p2.enter_context(tc.tile_pool(name="lp2", bufs=2))
            lph = p2.enter_context(tc.tile_pool(name="lph", bufs=2))
            ps_t = p2.enter_context(tc.tile_pool(name="ps_t", bufs=2, space="PSUM"))
            ps_fp = p2.enter_context(tc.tile_pool(name="ps_fp", bufs=2, space="PSUM"))
            ps_f2 = p2.enter_context(tc.tile_pool(name="ps_f2", bufs=2, space="PSUM"))
            ps_row = p2.enter_context(tc.tile_pool(name="ps_row", bufs=1, space="PSUM"))

            def stage_A(t):
                sl = slice(t * 512, (t + 1) * 512)
                z1res = lp2.tile([128, CC, 512], BF16, tag="z1res", name="z1res")
                nc.sync.dma_start(z1res[:], z1rbr[:, :, sl])
                sqp = ps_row.tile([8, 512], F32, tag="aux", name="sqp")

                # q softmax denominators
                for pr in range(CC):
                    mm(sqp[:], selq[:, pr, :], qsm8[:, pr, sl],
                       start=(pr == 0), stop=(pr == CC - 1), skip_group_check=True)
                rq = lp2.tile([8, 512], F32R, tag="rq", name="rq")
                with nc.allow_low_precision(reason="rq rounds to f32r for bcast matmul"):
                    nc.vector.reciprocal(rq[:], sqp[:])

                # attention apply
                att8 = lp2.tile([128, CC, 512], F8, tag="att8", name="att8")
                rqbs = lp2.tile([128, CC, 512], BF16, tag="rqbs", bufs=1,
                                name="rqbs")
                for pr in range(CC):
                    rqb = ps_t.tile([128, 512], F32, tag="pst", name="rqb")
                    mm(rqb[:], selbc[:, pr, :], rq[:],
                       start=True, stop=True)
                    act(rqbs[:, pr, :], rqb[:], AFT.Copy)
                    attps = ps_t.tile([128, 512], F32, tag="pst", name="attps")
                    mm(attps[:], ctxbd[pr][:], qsm8[:, pr, sl],
                       start=True, stop=True)
                    tt(att8[:, pr, :], attps[:], rqbs[:, pr, :], AluOpType.mult)

                # z = Wr att + z1 + biases
                zt = lp2.tile([128, CC, 512], F32R, tag="zt", bufs=1, name="zt")
                zsq = lp2.tile([128, CC, 512], BF16, tag="zsq", bufs=1, name="zsq")
                for oc in range(CC):
                    zps = ps_t.tile([128, 512], F32, tag="pst", name="zps")
                    for p in range(2):
                        mm(zps[:], wr8[:, 2 * p:2 * p + 2, oc * 128:(oc + 1) * 128],
                           att8[:, 2 * p:2 * p + 2, :],
                           start=(p == 0), stop=(p == 1), perf_mode=DR)
                    stt(zt[:, oc, :], zps[:], cst["c_z"],
                        z1res[:, oc, :], AluOpType.mult, AluOpType.add)
                    act(zsq[:, oc, :], zt[:, oc, :].bitcast(F32), AFT.Square)
                # LN1 stats: mu then e2 through one rotating psum bank
                mu1p = ps_row.tile([1, 512], F32, tag="rowacc", name="mu1p")
                for oc in range(CC):
                    mm(mu1p[:], inv512[:], zt[:, oc, :],
                       start=(oc == 0), stop=(oc == CC - 1), skip_group_check=True)
                mu1row = lp2.tile([1, 512], F32, tag="mu1row", name="mu1row")
                act(mu1row[:], mu1p[:], AFT.Copy)
                e2p = ps_row.tile([1, 512], F32, tag="rowacc", name="e2p")
                for oc in range(CC):
                    mm(e2p[:], inv512b[:], zsq[:, oc, :],
                       start=(oc == 0), stop=(oc == CC - 1), skip_group_check=True)

                # LN1 rows: rsig1 = exp(-0.5 ln(var+eps))
                musq1 = lp2.tile([1, 512], F32, tag="musq1", name="musq1")
                act(musq1[:], mu1row[:], AFT.Square)
                var1 = lp2.tile([1, 512], F32, tag="var1", name="var1")
                tt(var1[:], e2p[:], musq1[:], AluOpType.subtract)
                lnv1 = lp2.tile([1, 512], F32, tag="lnv1", name="lnv1")
                act(lnv1[:], var1[:], AFT.Ln, bias=misc[0:1, 1:2])
                rs1 = lp2.tile([1, 512], F32R, tag="rs1", name="rs1")
                act(rs1[:], lnv1[:], AFT.Exp, scale=-0.5)
                mneg = lp2.tile([1, 512], F32R, tag="mneg", name="mneg")
                tt(mneg[:], mu1row[:], rs1[:], AluOpType.mult)
                rs1b = ps_row.tile([128, 512], F32, tag="aux", name="rs1b")
                mm(rs1b[:], ones1x128[:], rs1[:], start=True, stop=True,
                   skip_group_check=True)

                # zn = z * rsig1
                zn = lp2.tile([128, CC, 512], F32R, tag="zn", name="zn")
                for oc in range(CC):
                    tt(zn[:, oc, :], zt[:, oc, :].bitcast(F32), rs1b[:],
                       AluOpType.mult)
                return sl, zn, mneg

            def stage_B(st):
                sl, zn, mneg = st

                # FFN1 + ELU
                he2 = lph.tile([128, 8, 512], F32R, tag="he2", bufs=1, name="he2")
                for j in range(8):
                    js = slice(j * 128, (j + 1) * 128)
                    fps = ps_fp.tile([128, 512], F32, tag="fps", name="fps")
                    for cc2 in range(CC):
                        mm(fps[:], w1g[:, cc2, js], zn[:, cc2, :],
                           start=(cc2 == 0), stop=False)
                    mm(fps[:], u1wa[:, js], mneg[:], start=False,
                       stop=cst["w1bb0"] > 0.5, skip_group_check=True)
                    if not cst["w1bb0"] > 0.5:
                        mm(fps[:], u1wb[:, js], ones512[:], start=False,
                           stop=True, skip_group_check=True)
                    E = lp2.tile([128, 512], BF16, tag="E", bufs=2, name="E")
                    act(E[:], fps[:], AFT.Exp)
                    t2 = lp2.tile([128, 512], BF16, tag="t2", bufs=2, name="t2")
                    nc.gpsimd.tensor_scalar(t2[:], E[:], 1.0, None, AluOpType.min)
                    stt(he2[:, j, :], fps[:], 0.0, t2[:], AluOpType.max,
                        AluOpType.add)
                # LN2 mean rows (u2c . he2)
                mu2p = ps_row.tile([1, 512], F32, tag="rowacc", name="mu2p")
                for j in range(8):
                    mm(mu2p[:], u2c[:, j:j + 1], he2[:, j, :],
                       start=(j == 0), stop=(j == 7), skip_group_check=True)
                mu2row = lp2.tile([1, 512], F32, tag="mu2row", name="mu2row")
                act(mu2row[:], mu2p[:], AFT.Copy)
                e2yp = ps_row.tile([1, 512], F32, tag="rowacc", name="e2yp")

                # FFN2 o-outer + LN2 stats
                g2y = lph.tile([128, CC, 512], F32R, tag="g2y", bufs=1, name="g2y")
                for o2 in range(CC):
                    os_ = slice(o2 * 128, (o2 + 1) * 128)
                    f2ps = ps_f2.tile([128, 512], F32, tag="f2", name="f2ps")
                    for j in range(8):
                        mm(f2ps[:], w2c[:, j, os_], he2[:, j, :],
                           start=(j == 0), stop=(j == 7))
                    ysq = lp2.tile([128, 512], F32R, tag="ysq", name="ysq")
                    act(ysq[:], f2ps[:], AFT.Square,
                        bias=ybc[:, o2:o2 + 1], scale=ivg2[:, o2:o2 + 1])
                    mm(e2yp[:], inv512[:], ysq[:],
                       start=(o2 == 0), stop=(o2 == CC - 1), skip_group_check=True)
                    act(g2y[:, o2, :], f2ps[:], AFT.Identity,
                        bias=gyb[:, o2:o2 + 1])

                # LN2 rows
                negmu2 = lp2.tile([1, 512], F32R, tag="negmu2", name="negmu2")
                ts(negmu2[:], mu2row[:], cst["kc"], -1.0,
                   AluOpType.add, AluOpType.mult)
                musq2 = lp2.tile([1, 512], F32, tag="musq2", name="musq2")
                tt(musq2[:], negmu2[:].bitcast(F32), negmu2[:].bitcast(F32),
                   AluOpType.mult)
                var2 = lp2.tile([1, 512], F32, tag="var2", name="var2")
                tt(var2[:], e2yp[:], musq2[:], AluOpType.subtract)
                lnv2 = lp2.tile([1, 512], F32, tag="lnv2", name="lnv2")
                act(lnv2[:], var2[:], AFT.Ln, bias=misc[0:1, 1:2])
                rs2 = lp2.tile([1, 512], F32R, tag="rs2", name="rs2")
                act(rs2[:], lnv2[:], AFT.Exp, scale=-0.5)
                sig2 = lp2.tile([1, 512], F32R, tag="sig2", name="sig2")
                act(sig2[:], lnv2[:], AFT.Exp, scale=0.5)
                rs2b = ps_row.tile([128, 512], F32, tag="aux", name="rs2b")
                mm(rs2b[:], ones1x128[:], rs2[:], start=True, stop=True,
                   skip_group_check=True)

                # finalize: out = (g2y + g2(x)negmu2 + be2(x)sig2) * rsig2
                outt = lp2.tile([128, CC, 512], F32, tag="outt", bufs=1, name="outt")
                for o2 in range(CC):
                    os_ = slice(o2 * 128, (o2 + 1) * 128)
                    Rp = ps_t.tile([128, 512], F32, tag="pst", name="Rp")
                    mm(Rp[:], g3g[:, os_], nmrs[:], start=True, stop=True)
                    w_ = lp2.tile([128, 512], F32, tag="w_", name="w_")
                    tt(w_[:], g2y[:, o2, :].bitcast(F32), rs2b[:], AluOpType.mult)
                    stt(outt[:, o2, :], w_[:], be2c[:, o2:o2 + 1], Rp[:],
                        AluOpType.add, AluOpType.add)
                nc.sync.dma_start(outr[:, :, sl], outt[:])

            stA = stage_A(0)
            for t in range(1, NT):
                stA_next = stage_A(t)
                stage_B(stA)
                stA = stA_next
            stage_B(stA)

    nc.compile()
    return nc


def _prep_consts(z1, z2, Wq, bq, Wk, bk, Wv, bv, Wr, br, g1, be1,
                 W1, b1, W2, b2, g2, be2):
    import ml_dtypes
    f = np.float32
    F8NP = ml_dtypes.float8_e4m3

    def wscale(W):
        m = float(np.abs(W).max())
        return 160.0 / m if m > 0 else 1.0

    s1 = 160.0 / max(float(np.abs(z1).max()), 1e-30)
    s2 = 160.0 / max(float(np.abs(z2).max()), 1e-30)
    swq, swk, swv, swr = wscale(Wq), wscale(Wk), wscale(Wv), wscale(Wr)
    s_q, s_e, s_v, s_c, s_a = 4.0, 4.0, 32.0, 32.0, 32.0

    def chunkT(a, s):          # [out, in] -> [128, in//128, out] scaled fp8
        o, cin = a.shape
        return np.ascontiguousarray(
            (a.T * s).reshape(cin // 128, 128, o).transpose(1, 0, 2)).astype(F8NP)

    wq8 = chunkT(Wq, swq)                                       # [128, 4, 512]
    wkv = np.concatenate([Wk.T * swk, Wv.T * swv], axis=1)      # [512, 1024]
    wkv8 = np.ascontiguousarray(
        wkv.reshape(CC, 128, 1024).transpose(1, 0, 2)).astype(F8NP)
    wr8 = chunkT(Wr, swr)                                       # [128, 4, 512]

    W1g = (W1 * g1[None, :]).astype(f)                          # [1024, 512]
    w1g = np.ascontiguousarray(
        W1g.T.reshape(CC, 128, 1024).transpose(1, 0, 2)).astype(f)
    u1 = W1g.sum(axis=1).astype(f)
    w1bb = (W1 @ be1 + b1).astype(f)
    u1wa = (-u1).reshape(1, 1024).astype(f)
    u1wb = w1bb.reshape(1, 1024).astype(f)

    W2g = (W2 * g2[:, None]).astype(f)                          # [512, 1024]
    w2c = np.ascontiguousarray(
        W2g.T.reshape(8, 128, 512).transpose(1, 0, 2)).astype(f)
    u2 = (W2.sum(axis=0) / 512.0).astype(f)                     # [1024]
    u2c = np.ascontiguousarray(u2.reshape(8, 128).T).astype(f)  # [128, 8]
    w2sum = W2.sum(axis=1).astype(f)                            # [512]
    kc = float(np.mean(b2) - np.mean(w2sum))
    g3g = g2.reshape(1, 512).astype(f)
    g3be = be2.reshape(1, 512).astype(f)

    def colsC(v):
        return np.ascontiguousarray(np.asarray(v, f).reshape(CC, 128).T)

    selq = np.zeros((128, CC, 8), dtype=F8NP)
    selbc = np.zeros((8, CC, 128), dtype=f)
    for pr in range(CC):
        for hb in range(2):
            h = 2 * pr + hb
            selq[hb * 64:(hb + 1) * 64, pr, h] = 1.0
            selbc[h, pr, hb * 64:(hb + 1) * 64] = s_a / s_c

    consts = {
        "wq8": wq8, "wkv8": wkv8, "wr8": wr8,
        "w1g": w1g, "u1wa": u1wa, "u1wb": u1wb, "w2c": w2c, "u2c": u2c,
        "g3g": g3g, "g3be": g3be,
        "ones512": np.ones((1, 512), dtype=f),
        "selq": selq, "selbc": selbc,
        "eqb": colsC(bq + np.log(s_q)),
        "ivg2": colsC(1.0 / g2),
        "ybc": colsC(b2 - w2sum),
        "gyb": colsC(g2 * (b2 - w2sum)),
        "be2c": colsC(be2),
        "inv512": np.full((128, 1), 1.0 / 512.0, dtype=f),
        "inv512b": np.full((128, 1), 1.0 / 512.0, dtype=np.dtype("bfloat16") if False else f).astype(__import__("ml_dtypes").bfloat16),
        "ones1x128": np.ones((1, 128), dtype=f),
        "misc": np.stack([np.full(128, np.log(s_e), dtype=f), np.full(128, EPS, dtype=f)], axis=1),
    }
    scalars = {
        "c_q": 1.0 / (swq * s1),
        "c_k": 1.0 / (swk * s2),
        "ln_se": float(np.log(s_e)),
        "c_v": s_v / (swv * s2),
        "s_cv": s_c / s_v,
        "c_z": 1.0 / (swr * s_a),
        "kc": kc,
        "w1bb0": 1.0 if not np.any(w1bb) else 0.0,
    }
    import ml_dtypes as mld
    z1rb = (z1 + (br + Wr @ bv)[None, :, None]).astype(mld.bfloat16)
    return consts, scalars, z1rb, s1, s2


def kernel(**inputs):
    global LAST_RESULT, CONSTS
    import ml_dtypes
    F8NP = ml_dtypes.float8_e4m3
    z1 = np.asarray(inputs["z1"], dtype=np.float32)
    z2 = np.asarray(inputs["z2"], dtype=np.float32)
    wargs = [np.asarray(inputs[k], dtype=np.float32) for k in
             ["Wq", "bq", "Wk", "bk", "Wv", "bv", "Wr", "br", "g1", "be1",
              "W1", "b1", "W2", "b2", "g2", "be2"]]
    consts, scalars, z1rb, s1, s2 = _prep_consts(z1, z2, *wargs)

    key = ("prog",) + tuple(sorted((k, round(v * 1e9)) for k, v in scalars.items()))
    if key not in _CACHE:
        CONSTS = scalars
        _CACHE.clear()
        _CACHE[key] = _build_program()
    nc = _CACHE[key]

    z1q = np.ascontiguousarray((z1 * s1).astype(F8NP))
    z2q = np.ascontiguousarray((z2 * s2).astype(F8NP))

    in_maps = []
    for b in range(B):
        m = dict(consts)
        m["z1q"] = z1q[b]
        m["z2q"] = z2q[b]
        m["z1rb"] = np.ascontiguousarray(z1rb[b])
        in_maps.append(m)

    import os
    trace = bool(int(os.environ.get("KERNEL_TRACE", "0")))
    res = run_bass_kernel_spmd(nc, in_maps, list(range(B)), trace=trace)
    LAST_RESULT = res
    out = np.stack([res.results[b]["out"] for b in range(B)], axis=0)
    return out.astype(np.float32)


# revision 35
# speedup vs baseline: 2.2770x; 1.0280x over previous
"""CACombiner Trainium2 kernel: conv-projected efficient attention + FFN.

Data-parallel over batch: 8 batch elements -> 8 NeuronCores, identical SPMD
program per core. Attention block (q/k/v, ctx, apply, reprojection) runs in
fp8e4 with DoubleRow matmuls; the FFN runs in float32r. LayerNorm rsqrt is
computed as exp(-0.5*ln(var+eps)) so every activation stays in one ACT table.
"""
import sys
sys.path.insert(0, "/opt/trn_rl_repo")
from contextlib import ExitStack

import numpy as np

import concourse.bass as bass
import concourse.tile as tile
from concourse import mybir, bacc
from concourse.bass_utils import run_bass_kernel_spmd
from concourse.alu_op_type import AluOpType

F32 = mybir.dt.float32
F32R = mybir.dt.float32r
BF16 = mybir.dt.bfloat16
F8 = mybir.dt.float8e4
AFT = mybir.ActivationFunctionType
DR = mybir.MatmulPerfMode.DoubleRow

B, C, L = 8, 512, 4096
H, DK = 8, 64
EPS = 1e-5
CC = C // 128           # 4 channel chunks
NT = L // 512           # 8 l-tiles (512 wide)

_CACHE = {}
LAST_RESULT = None

# compile-time floats baked into the program (set before _build_program)
CONSTS = {}


def _build_program():
    cst = CONSTS
    nc = bacc.Bacc("TRN2", target_bir_lowering=False, debug=False)

    def din(name, shape, dtype):
        return nc.dram_tensor(name, list(shape), dtype, kind="ExternalInput").ap()

    z1q_d = din("z1q", (C, L), F8)
    z2q_d = din("z2q", (C, L), F8)
    z1rb_d = din("z1rb", (C, L), BF16)
    wq8_d = din("wq8", (128, CC, 512), F8)
    wkv8_d = din("wkv8", (128, CC, 1024), F8)
    wr8_d = din("wr8", (128, CC, 512), F8)
    w1g_d = din("w1g", (128, CC, 1024), F32R)
    u1wa_d = din("u1wa", (1, 1024), F32R)
    u1wb_d = din("u1wb", (1, 1024), F32R)
    w2c_d = din("w2c", (128, 8, 512), F32R)
    u2c_d = din("u2c", (128, 8), F32R)
    g3g_d = din("g3g", (1, 512), F32R)
    g3be_d = din("g3be", (1, 512), F32R)
    ones512_d = din("ones512", (1, 512), F32R)
    selq_d = din("selq", (128, CC, 8), F8)
    selbc_d = din("selbc", (8, CC, 128), F32R)
    eqb_d = din("eqb", (128, CC), F32)
    ivg2_d = din("ivg2", (128, CC), F32)
    ybc_d = din("ybc", (128, CC), F32)
    gyb_d = din("gyb", (128, CC), F32)
    be2c_d = din("be2c", (128, CC), F32)
    inv512_d = din("inv512", (128, 1), F32R)
    inv512b_d = din("inv512b", (128, 1), BF16)
    ones1x128_d = din("ones1x128", (1, 128), F32R)
    misc_d = din("misc", (128, 4), F32)
    outd = nc.dram_tensor("out", [C, L], F32, kind="ExternalOutput").ap()

    z1qr = z1q_d.rearrange("(cc p) l -> p cc l", p=128)
    z2qr = z2q_d.rearrange("(cc p) l -> p cc l", p=128)
    z1rbr = z1rb_d.rearrange("(cc p) l -> p cc l", p=128)
    outr = outd.rearrange("(cc p) l -> p cc l", p=128)

    mm = nc.tensor.matmul
    tt = nc.vector.tensor_tensor
    ts = nc.vector.tensor_scalar
    stt = nc.vector.scalar_tensor_tensor
    act = nc.scalar.activation

    with tile.TileContext(nc) as tc, ExitStack() as ctx:
        cpool = ctx.enter_context(tc.tile_pool(name="consts", bufs=1))

        def const_tile(shape, dtype, src, tag):
            t = cpool.tile(list(shape), dtype, tag=tag, name=tag)
            nc.sync.dma_start(t[:], src)
            return t

        # phase-1 weights + small consts first (so phase 1 starts fast)
        wq8 = const_tile((128, CC, 512), F8, wq8_d, "wq8")
        wkv8 = const_tile((128, CC, 1024), F8, wkv8_d, "wkv8")
        eqb = const_tile((128, CC), F32, eqb_d, "eqb")
        misc = const_tile((128, 4), F32, misc_d, "misc")

        # phase-2-only consts: tiles now, DMAs deferred into the phase-1 loop
        def const_tile_nodma(shape, dtype, tag):
            return cpool.tile(list(shape), dtype, tag=tag, name=tag)

        wr8 = const_tile_nodma((128, CC, 512), F8, "wr8")
        selq = const_tile_nodma((128, CC, 8), F8, "selq")
        selbc = const_tile_nodma((8, CC, 128), F32R, "selbc")
        ivg2 = const_tile_nodma((128, CC), F32, "ivg2")
        ybc = const_tile_nodma((128, CC), F32, "ybc")
        gyb = const_tile_nodma((128, CC), F32, "gyb")
        be2c = const_tile_nodma((128, CC), F32, "be2c")
        inv512 = const_tile_nodma((128, 1), F32R, "inv512")
        inv512b = const_tile_nodma((128, 1), BF16, "inv512b")
        ones1x128 = const_tile_nodma((1, 128), F32R, "ones1x128")
        u1wa = const_tile_nodma((1, 1024), F32R, "u1wa")
        if not cst["w1bb0"] > 0.5:
            u1wb = const_tile_nodma((1, 1024), F32R, "u1wb")
            ones512 = const_tile_nodma((1, 512), F32R, "ones512")
        u2c = const_tile_nodma((128, 8), F32R, "u2c")
        g3g = const_tile_nodma((1, 512), F32R, "g3g")
        g3be = const_tile_nodma((1, 512), F32R, "g3be")

        def load_deferred_consts():
            nc.sync.dma_start(wr8[:], wr8_d)
            nc.sync.dma_start(selq[:], selq_d)
            nc.sync.dma_start(selbc[:], selbc_d)
            nc.sync.dma_start(ivg2[:], ivg2_d)
            nc.sync.dma_start(ybc[:], ybc_d)
            nc.sync.dma_start(gyb[:], gyb_d)
            nc.sync.dma_start(be2c[:], be2c_d)
            nc.sync.dma_start(inv512[:], inv512_d)
            nc.sync.dma_start(inv512b[:], inv512b_d)
            nc.sync.dma_start(ones1x128[:], ones1x128_d)
            nc.sync.dma_start(u1wa[:], u1wa_d)
            if not cst["w1bb0"] > 0.5:
                nc.sync.dma_start(u1wb[:], u1wb_d)
                nc.sync.dma_start(ones512[:], ones512_d)
            nc.sync.dma_start(u2c[:], u2c_d)
            nc.sync.dma_start(g3g[:], g3g_d)
            nc.sync.dma_start(g3be[:], g3be_d)

        # big FFN weights: DMA issued inside the phase-1 loop (overlap)
        w1g = cpool.tile([128, CC, 1024], F32R, tag="w1g", name="w1g")
        w2c = cpool.tile([128, 8, 512], F32R, tag="w2c", name="w2c")

        # persistent across phases
        qsm8 = cpool.tile([128, CC, L], F8, tag="qsm8", name="qsm8")
        ctxbd = [cpool.tile([128, 128], F8, tag=f"ctxbd{p}", name=f"ctxbd{p}")
                 for p in range(CC)]
        nc.scalar.add_instruction(mybir.InstLoadActFuncSet(
            name=nc.get_next_instruction_name(), ins=[], outs=[],
            act_func_set_id=6))

        # ---------------- Phase 1: q/k/v + exp + ctx accumulation ----------------
        with ExitStack() as p1:
            lp1 = p1.enter_context(tc.tile_pool(name="lp1", bufs=2))
            ps_kv = p1.enter_context(tc.tile_pool(name="ps_kv", bufs=2, space="PSUM"))
            ps_q = p1.enter_context(tc.tile_pool(name="ps_q", bufs=2, space="PSUM"))
            ps_ctx = p1.enter_context(tc.tile_pool(name="ps_ctx", bufs=1, space="PSUM"))

            ctxps = [ps_ctx.tile([128, 2, 129], F32, tag=f"ctx{i}", name=f"ctxps{i}")
                     for i in range(2)]

            pend = None       # lagged ctx MM emission: (ek8, v8t, is_first)
            first_ctx = True

            def flush_ctx(last=False):
                nonlocal pend, first_ctx
                if pend is None:
                    return
                ek8_, v8t_, _ = pend
                for s01 in range(2):
                    for pr in range(CC):
                        mm(ctxps[pr // 2][:, pr % 2, :],
                           ek8_[:, s01, pr * 128:(pr + 1) * 128],
                           v8t_[:, s01, pr, :],
                           start=(first_ctx and s01 == 0),
                           stop=(last and s01 == 1),
                           skip_group_check=True)
                first_ctx = False
                pend = None

            for t in range(NT):
                sl = slice(t * 512, (t + 1) * 512)
                z1t = lp1.tile([128, CC, 512], F8, tag="z1t")
                nc.sync.dma_start(z1t[:], z1qr[:, :, sl])
                z2t = lp1.tile([128, CC, 512], F8, tag="z2t")
                nc.sync.dma_start(z2t[:], z2qr[:, :, sl])
                # stream the big FFN weights during phase 1
                if t == 1:
                    nc.sync.dma_start(w1g[:], w1g_d)
                elif t == 2:
                    nc.sync.dma_start(w2c[:], w2c_d)
                elif t == 3:
                    load_deferred_consts()

                # ---- q in [o, l] ----
                if cst["bq0"] > 0.5:
                    # joint exp over oc-pairs (uniform bias ln(s_q))
                    for op_ in range(2):
                        qps = ps_q.tile([128, 2, 512], F32, tag="qpp",
                                        bufs=1, name="qps")
                        for i in range(2):
                            oc = op_ * 2 + i
                            for p in range(2):
                                mm(qps[:, i, :],
                                   wq8[:, 2 * p:2 * p + 2,
                                       oc * 128:(oc + 1) * 128],
                                   z1t[:, 2 * p:2 * p + 2, :],
                                   start=(p == 0), stop=(p == 1), perf_mode=DR)
                        act(qsm8[:, op_ * 2:op_ * 2 + 2, sl], qps[:], AFT.Exp,
                            bias=misc[:, 2:3], scale=cst["c_q"])
                else:
                    for oc in range(CC):
                        qps = ps_q.tile([128, 512], F32, tag="qps", name="qps")
                        for p in range(2):
                            mm(qps[:],
                               wq8[:, 2 * p:2 * p + 2, oc * 128:(oc + 1) * 128],
                               z1t[:, 2 * p:2 * p + 2, :],
                               start=(p == 0), stop=(p == 1), perf_mode=DR)
                        act(qsm8[:, oc, sl], qps[:], AFT.Exp,
                            bias=eqb[:, oc:oc + 1], scale=cst["c_q"])

                # ---- k/v per 128-l subtile; ctx lagged one sub-pair ----
                for sp in range(2):
                    ek8 = lp1.tile([128, 2, 512], F8, tag="ek8")
                    v8t = lp1.tile([128, 2, CC, 129], F8, tag="v8t")
                    nc.vector.memset(v8t[:, :, :, 128:129], 1.0)
                    kpair = ps_kv.tile([128, 2, 512], F32, tag="kpair",
                                       bufs=1, name="kpair")
                    for s01 in range(2):
                        sub = sp * 2 + s01
                        ssl = slice(sub * 128, (sub + 1) * 128)
                        vps = ps_kv.tile([128, 512], F32, tag="vps", name="vps")
                        for p in range(2):
                            mm(kpair[:, s01, :],
                               z2t[:, 2 * p:2 * p + 2, ssl],
                               wkv8[:, 2 * p:2 * p + 2, 0:512],
                               start=(p == 0), stop=(p == 1), perf_mode=DR)
                        for p in range(2):
                            mm(vps[:],
                               z2t[:, 2 * p:2 * p + 2, ssl],
                               wkv8[:, 2 * p:2 * p + 2, 512:1024],
                               start=(p == 0), stop=(p == 1), perf_mode=DR)
                        ts(v8t[:, s01, :, 0:128],
                           vps[:].rearrange("p (cc x) -> p cc x", x=128),
                           cst["c_v"], None, AluOpType.mult)
                    act(ek8[:], kpair[:], AFT.Exp,
                        bias=misc[:, 0:1], scale=cst["c_k"])
                    flush_ctx()
                    pend = (ek8, v8t, None)
            flush_ctx(last=True)

            # ---- finalize ctx: normalize rows, build block-diagonal fp8 ----
            rsc = lp1.tile([128, 2, 2], F32, tag="rsc", bufs=1)
            for i in range(2):
                nc.vector.reciprocal(rsc[:, i, :], ctxps[i][:, :, 128])
            for pr in range(CC):
                nc.vector.memset(ctxbd[pr][:], 0.0)
                for hb in range(2):
                    hs = slice(hb * 64, (hb + 1) * 64)
                    ts(ctxbd[pr][hs, hs],
                       ctxps[pr // 2][hs, pr % 2, hs],
                       rsc[hs, pr // 2, pr % 2:pr % 2 + 1],
                       cst["s_cv"], AluOpType.mult, AluOpType.mult)

        # ---------------- Phase 2: apply + reprojection + LN/FFN ----------------
        with ExitStack() as p2:
            lp2 = p2.enter_context(tc.tile_pool(name="lp2", bufs=2))
            lph = p2.enter_context(tc.tile_pool(name="lph", bufs=2))
            ps_t = p2.enter_context(tc.tile_pool(name="ps_t", bufs=2, space="PSUM"))
            ps_fp = p2.enter_context(tc.tile_pool(name="ps_fp", bufs=2, space="PSUM"))
            ps_f2 = p2.enter_context(tc.tile_pool(name="ps_f2", bufs=2, space="PSUM"))
            ps_row = p2.enter_context(tc.tile_pool(name="ps_row", bufs=1, space="PSUM"))

            def stage_A(t):
                sl = slice(t * 512, (t + 1) * 512)
                z1res = lp2.tile([128, CC, 512], BF16, tag="z1res", name="z1res")
                nc.sync.dma_start(z1res[:], z1rbr[:, :, sl])
                sqp = ps_row.tile([8, 512], F32, tag="aux", name="sqp")

                # q softmax denominators
                for pr in range(CC):
                    mm(sqp[:], selq[:, pr, :], qsm8[:, pr, sl],
                       start=(pr == 0), stop=(pr == CC - 1), skip_group_check=True)
                rq = lp2.tile([8, 512], F32R, tag="rq", name="rq")
                with nc.allow_low_precision(reason="rq rounds to f32r for bcast matmul"):
                    nc.vector.reciprocal(rq[:], sqp[:])

                # attention apply
                att8 = lp2.tile([128, CC, 512], F8, tag="att8", name="att8")
                rqbs = lp2.tile([128, CC, 512], BF16, tag="rqbs", bufs=1,
                                name="rqbs")
                for pr in range(CC):
                    rqb = ps_t.tile([128, 512], F32, tag="pst", name="rqb")
                    mm(rqb[:], selbc[:, pr, :], rq[:],
                       start=True, stop=True)
                    act(rqbs[:, pr, :], rqb[:], AFT.Copy)
                    attps = ps_t.tile([128, 512], F32, tag="pst", name="attps")
                    mm(attps[:], ctxbd[pr][:], qsm8[:, pr, sl],
                       start=True, stop=True)
                    tt(att8[:, pr, :], attps[:], rqbs[:, pr, :], AluOpType.mult)

                # z = Wr att + z1 + biases
                zt = lp2.tile([128, CC, 512], F32R, tag="zt", bufs=1, name="zt")
                zsq = lp2.tile([128, CC, 512], BF16, tag="zsq", bufs=1, name="zsq")
                for oc in range(CC):
                    zps = ps_t.tile([128, 512], F32, tag="pst", name="zps")
                    for p in range(2):
                        mm(zps[:], wr8[:, 2 * p:2 * p + 2, oc * 128:(oc + 1) * 128],
                           att8[:, 2 * p:2 * p + 2, :],
                           start=(p == 0), stop=(p == 1), perf_mode=DR)
                    stt(zt[:, oc, :], zps[:], cst["c_z"],
                        z1res[:, oc, :], AluOpType.mult, AluOpType.add)
                    act(zsq[:, oc, :], zt[:, oc, :].bitcast(F32), AFT.Square)
                # LN1 stats: mu then e2 through one rotating psum bank
                mu1p = ps_row.tile([1, 512], F32, tag="rowacc", name="mu1p")
                for oc in range(CC):
                    mm(mu1p[:], inv512[:], zt[:, oc, :],
                       start=(oc == 0), stop=(oc == CC - 1), skip_group_check=True)
                mu1row = lp2.tile([1, 512], F32, tag="mu1row", name="mu1row")
                act(mu1row[:], mu1p[:], AFT.Copy)
                e2p = ps_row.tile([1, 512], F32, tag="rowacc", name="e2p")
                for oc in range(CC):
                    mm(e2p[:], inv512b[:], zsq[:, oc, :],
                       start=(oc == 0), stop=(oc == CC - 1), skip_group_check=True)

                # LN1 rows: rsig1 = exp(-0.5 ln(var+eps))
                musq1 = lp2.tile([1, 512], F32, tag="musq1", name="musq1")
                act(musq1[:], mu1row[:], AFT.Square)
                var1 = lp2.tile([1, 512], F32, tag="var1", name="var1")
                tt(var1[:], e2p[:], musq1[:], AluOpType.subtract)
                lnv1 = lp2.tile([1, 512], F32, tag="lnv1", name="lnv1")
                act(lnv1[:], var1[:], AFT.Ln, bias=misc[0:1, 1:2])
                rs1 = lp2.tile([1, 512], F32R, tag="rs1", name="rs1")
                act(rs1[:], lnv1[:], AFT.Exp, scale=-0.5)
                mneg = lp2.tile([1, 512], F32R, tag="mneg", name="mneg")
                tt(mneg[:], mu1row[:], rs1[:], AluOpType.mult)
                rs1b = ps_row.tile([128, 512], F32, tag="aux", name="rs1b")
                mm(rs1b[:], ones1x128[:], rs1[:], start=True, stop=True,
                   skip_group_check=True)

                # zn = z * rsig1
                zn = lp2.tile([128, CC, 512], F32R, tag="zn", name="zn")
                for oc in range(CC):
                    tt(zn[:, oc, :], zt[:, oc, :].bitcast(F32), rs1b[:],
                       AluOpType.mult)
                return sl, zn, mneg

            def stage_B(st):
                sl, zn, mneg = st

                # FFN1 + ELU
                he2 = lph.tile([128, 8, 512], F32R, tag="he2", bufs=1, name="he2")
                for j in range(8):
                    js = slice(j * 128, (j + 1) * 128)
                    fps = ps_fp.tile([128, 512], F32, tag="fps", name="fps")
                    for cc2 in range(CC):
                        mm(fps[:], w1g[:, cc2, js], zn[:, cc2, :],
                           start=(cc2 == 0), stop=False)
                    mm(fps[:], u1wa[:, js], mneg[:], start=False,
                       stop=cst["w1bb0"] > 0.5, skip_group_check=True)
                    if not cst["w1bb0"] > 0.5:
                        mm(fps[:], u1wb[:, js], ones512[:], start=False,
                           stop=True, skip_group_check=True)
                    E = lp2.tile([128, 512], BF16, tag="E", bufs=2, name="E")
                    act(E[:], fps[:], AFT.Exp)
                    t2 = lp2.tile([128, 512], BF16, tag="t2", bufs=2, name="t2")
                    nc.gpsimd.tensor_scalar(t2[:], E[:], 1.0, None, AluOpType.min)
                    stt(he2[:, j, :], fps[:], 0.0, t2[:], AluOpType.max,
                        AluOpType.add)
                # LN2 mean rows (u2c . he2)
                mu2p = ps_row.tile([1, 512], F32, tag="rowacc", name="mu2p")
                for j in range(8):
                    mm(mu2p[:], u2c[:, j:j + 1], he2[:, j, :],
                       start=(j == 0), stop=(j == 7), skip_group_check=True)
                mu2row = lp2.tile([1, 512], F32, tag="mu2row", name="mu2row")
                act(mu2row[:], mu2p[:], AFT.Copy)
                e2yp = ps_row.tile([1, 512], F32, tag="rowacc", name="e2yp")

                # FFN2 o-outer + LN2 stats
                g2y = lph.tile([128, CC, 512], F32R, tag="g2y", bufs=1, name="g2y")
                for o2 in range(CC):
                    os_ = slice(o2 * 128, (o2 + 1) * 128)
                    f2ps = ps_f2.tile([128, 512], F32, tag="f2", name="f2ps")
                    for j in range(8):
                        mm(f2ps[:], w2c[:, j, os_], he2[:, j, :],
                           start=(j == 0), stop=(j == 7))
                    ysq = lp2.tile([128, 512], F32R, tag="ysq", name="ysq")
                    act(ysq[:], f2ps[:], AFT.Square,
                        bias=ybc[:, o2:o2 + 1], scale=ivg2[:, o2:o2 + 1])
                    mm(e2yp[:], inv512[:], ysq[:],
                       start=(o2 == 0), stop=(o2 == CC - 1), skip_group_check=True)
                    act(g2y[:, o2, :], f2ps[:], AFT.Identity,
                        bias=gyb[:, o2:o2 + 1])

                # LN2 rows
                negmu2 = lp2.tile([1, 512], F32R, tag="negmu2", name="negmu2")
                ts(negmu2[:], mu2row[:], cst["kc"], -1.0,
                   AluOpType.add, AluOpType.mult)
                musq2 = lp2.tile([1, 512], F32, tag="musq2", name="musq2")
                tt(musq2[:], negmu2[:].bitcast(F32), negmu2[:].bitcast(F32),
                   AluOpType.mult)
                var2 = lp2.tile([1, 512], F32, tag="var2", name="var2")
                tt(var2[:], e2yp[:], musq2[:], AluOpType.subtract)
                lnv2 = lp2.tile([1, 512], F32, tag="lnv2", name="lnv2")
                act(lnv2[:], var2[:], AFT.Ln, bias=misc[0:1, 1:2])
                rs2 = lp2.tile([1, 512], F32R, tag="rs2", name="rs2")
                act(rs2[:], lnv2[:], AFT.Exp, scale=-0.5)
                sig2 = lp2.tile([1, 512], F32R, tag="sig2", name="sig2")
                act(sig2[:], lnv2[:], AFT.Exp, scale=0.5)
                rs2b = ps_row.tile([128, 512], F32, tag="aux", name="rs2b")
                mm(rs2b[:], ones1x128[:], rs2[:], start=True, stop=True,
                   skip_group_check=True)

                # finalize: out = (g2y + g2(x)negmu2 + be2(x)sig2) * rsig2
                outt = lp2.tile([128, CC, 512], F32, tag="outt", bufs=1, name="outt")
                for o2 in range(CC):
                    os_ = slice(o2 * 128, (o2 + 1) * 128)
                    Rp = ps_t.tile([128, 512], F32, tag="pst", name="Rp")
                    mm(Rp[:], g3g[:, os_], nmrs[:], start=True, stop=True)
                    w_ = lp2.tile([128, 512], F32, tag="w_", name="w_")
                    tt(w_[:], g2y[:, o2, :].bitcast(F32), rs2b[:], AluOpType.mult)
                    stt(outt[:, o2, :], w_[:], be2c[:, o2:o2 + 1], Rp[:],
                        AluOpType.add, AluOpType.add)
                nc.sync.dma_start(outr[:, :, sl], outt[:])

            stA = stage_A(0)
            for t in range(1, NT):
                stA_next = stage_A(t)
                stage_B(stA)
                stA = stA_next
            stage_B(stA)

    nc.compile()
    return nc


def _prep_consts(z1, z2, Wq, bq, Wk, bk, Wv, bv, Wr, br, g1, be1,
                 W1, b1, W2, b2, g2, be2):
    import ml_dtypes
    f = np.float32
    F8NP = ml_dtypes.float8_e4m3

    def wscale(W):
        m = float(np.abs(W).max())
        return 160.0 / m if m > 0 else 1.0

    s1 = 160.0 / max(float(np.abs(z1).max()), 1e-30)
    s2 = 160.0 / max(float(np.abs(z2).max()), 1e-30)
    swq, swk, swv, swr = wscale(Wq), wscale(Wk), wscale(Wv), wscale(Wr)
    s_q, s_e, s_v, s_c, s_a = 4.0, 4.0, 32.0, 32.0, 32.0

    def chunkT(a, s):          # [out, in] -> [128, in//128, out] scaled fp8
        o, cin = a.shape
        return np.ascontiguousarray(
            (a.T * s).reshape(cin // 128, 128, o).transpose(1, 0, 2)).astype(F8NP)

    wq8 = chunkT(Wq, swq)                                       # [128, 4, 512]
    wkv = np.concatenate([Wk.T * swk, Wv.T * swv], axis=1)      # [512, 1024]
    wkv8 = np.ascontiguousarray(
        wkv.reshape(CC, 128, 1024).transpose(1, 0, 2)).astype(F8NP)
    wr8 = chunkT(Wr, swr)                                       # [128, 4, 512]

    W1g = (W1 * g1[None, :]).astype(f)                          # [1024, 512]
    w1g = np.ascontiguousarray(
        W1g.T.reshape(CC, 128, 1024).transpose(1, 0, 2)).astype(f)
    u1 = W1g.sum(axis=1).astype(f)
    w1bb = (W1 @ be1 + b1).astype(f)
    u1wa = (-u1).reshape(1, 1024).astype(f)
    u1wb = w1bb.reshape(1, 1024).astype(f)

    W2g = (W2 * g2[:, None]).astype(f)                          # [512, 1024]
    w2c = np.ascontiguousarray(
        W2g.T.reshape(8, 128, 512).transpose(1, 0, 2)).astype(f)
    u2 = (W2.sum(axis=0) / 512.0).astype(f)                     # [1024]
    u2c = np.ascontiguousarray(u2.reshape(8, 128).T).astype(f)  # [128, 8]
    w2sum = W2.sum(axis=1).astype(f)                            # [512]
    kc = float(np.mean(b2) - np.mean(w2sum))
    g3g = g2.reshape(1, 512).astype(f)
    g3be = be2.reshape(1, 512).astype(f)

    def colsC(v):
        return np.ascontiguousarray(np.asarray(v, f).reshape(CC, 128).T)

    selq = np.zeros((128, CC, 8), dtype=F8NP)
    selbc = np.zeros((8, CC, 128), dtype=f)
    for pr in range(CC):
        for hb in range(2):
            h = 2 * pr + hb
            selq[hb * 64:(hb + 1) * 64, pr, h] = 1.0
            selbc[h, pr, hb * 64:(hb + 1) * 64] = s_a / s_c

    consts = {
        "wq8": wq8, "wkv8": wkv8, "wr8": wr8,
        "w1g": w1g, "u1wa": u1wa, "u1wb": u1wb, "w2c": w2c, "u2c": u2c,
        "g3g": g3g, "g3be": g3be,
        "ones512": np.ones((1, 512), dtype=f),
        "selq": selq, "selbc": selbc,
        "eqb": colsC(bq + np.log(s_q)),
        "ivg2": colsC(1.0 / g2),
        "ybc": colsC(b2 - w2sum),
        "gyb": colsC(g2 * (b2 - w2sum)),
        "be2c": colsC(be2),
        "inv512": np.full((128, 1), 1.0 / 512.0, dtype=f),
        "inv512b": np.full((128, 1), 1.0 / 512.0, dtype=np.dtype("bfloat16") if False else f).astype(__import__("ml_dtypes").bfloat16),
        "ones1x128": np.ones((1, 128), dtype=f),
        "misc": np.stack([np.full(128, np.log(s_e), dtype=f), np.full(128, EPS, dtype=f), np.full(128, np.log(s_q), dtype=f), np.full(128, kc, dtype=f)], axis=1),
    }
    scalars = {
        "c_q": 1.0 / (swq * s1),
        "c_k": 1.0 / (swk * s2),
        "ln_se": float(np.log(s_e)),
        "c_v": s_v / (swv * s2),
        "s_cv": s_c / s_v,
        "c_z": 1.0 / (swr * s_a),
        "kc": kc,
        "w1bb0": 1.0 if not np.any(w1bb) else 0.0,
        "bq0": 1.0 if not np.any(bq) else 0.0,
    }
    import ml_dtypes as mld
    z1rb = (z1 + (br + Wr @ bv)[None, :, None]).astype(mld.bfloat16)
    return consts, scalars, z1rb, s1, s2


def kernel(**inputs):
    global LAST_RESULT, CONSTS
    import ml_dtypes
    F8NP = ml_dtypes.float8_e4m3
    z1 = np.asarray(inputs["z1"], dtype=np.float32)
    z2 = np.asarray(inputs["z2"], dtype=np.float32)
    wargs = [np.asarray(inputs[k], dtype=np.float32) for k in
             ["Wq", "bq", "Wk", "bk", "Wv", "bv", "Wr", "br", "g1", "be1",
              "W1", "b1", "W2", "b2", "g2", "be2"]]
    consts, scalars, z1rb, s1, s2 = _prep_consts(z1, z2, *wargs)

    key = ("prog",) + tuple(sorted((k, round(v * 1e9)) for k, v in scalars.items()))
    if key not in _CACHE:
        CONSTS = scalars
        _CACHE.clear()
        _CACHE[key] = _build_program()
    nc = _CACHE[key]

    z1q = np.ascontiguousarray((z1 * s1).astype(F8NP))
    z2q = np.ascontiguousarray((z2 * s2).astype(F8NP))

    in_maps = []
    for b in range(B):
        m = dict(consts)
        m["z1q"] = z1q[b]
        m["z2q"] = z2q[b]
        m["z1rb"] = np.ascontiguousarray(z1rb[b])
        in_maps.append(m)

    import os
    trace = bool(int(os.environ.get("KERNEL_TRACE", "0")))
    res = run_bass_kernel_spmd(nc, in_maps, list(range(B)), trace=trace)
    LAST_RESULT = res
    out = np.stack([res.results[b]["out"] for b in range(B)], axis=0)
    return out.astype(np.float32)


# revision 38
# speedup vs baseline: 2.2870x; 1.0044x over previous
"""CACombiner Trainium2 kernel: conv-projected efficient attention + FFN.

Data-parallel over batch: 8 batch elements -> 8 NeuronCores, identical SPMD
program per core. Attention block (q/k/v, ctx, apply, reprojection) runs in
fp8e4 with DoubleRow matmuls; the FFN runs in float32r. LayerNorm rsqrt is
computed as exp(-0.5*ln(var+eps)) so every activation stays in one ACT table.
"""
import sys
sys.path.insert(0, "/opt/trn_rl_repo")
from contextlib import ExitStack

import numpy as np

import concourse.bass as bass
import concourse.tile as tile
from concourse import mybir, bacc
from concourse.bass_utils import run_bass_kernel_spmd
from concourse.alu_op_type import AluOpType

F32 = mybir.dt.float32
F32R = mybir.dt.float32r
BF16 = mybir.dt.bfloat16
F8 = mybir.dt.float8e4
AFT = mybir.ActivationFunctionType
DR = mybir.MatmulPerfMode.DoubleRow

B, C, L = 8, 512, 4096
H, DK = 8, 64
EPS = 1e-5
CC = C // 128           # 4 channel chunks
NT = L // 512           # 8 l-tiles (512 wide)

_CACHE = {}
LAST_RESULT = None

# compile-time floats baked into the program (set before _build_program)
CONSTS = {}


def _build_program():
    cst = CONSTS
    nc = bacc.Bacc("TRN2", target_bir_lowering=False, debug=False)

    def din(name, shape, dtype):
        return nc.dram_tensor(name, list(shape), dtype, kind="ExternalInput").ap()

    z1q_d = din("z1q", (C, L), F8)
    z2q_d = din("z2q", (C, L), F8)
    z1rb_d = din("z1rb", (C, L), BF16)
    wq8_d = din("wq8", (128, CC, 512), F8)
    wkv8_d = din("wkv8", (128, CC, 1024), F8)
    wr8_d = din("wr8", (128, CC, 512), F8)
    w1g_d = din("w1g", (128, CC, 1024), F32R)
    u1wa_d = din("u1wa", (1, 1024), F32R)
    u1wb_d = din("u1wb", (1, 1024), F32R)
    w2c_d = din("w2c", (128, 8, 512), F32R)
    u2c_d = din("u2c", (128, 8), F32R)
    g3g_d = din("g3g", (1, 512), F32R)
    g3be_d = din("g3be", (1, 512), F32R)
    ones512_d = din("ones512", (1, 512), F32R)
    selq_d = din("selq", (128, CC, 8), F8)
    selbc_d = din("selbc", (8, CC, 128), F32R)
    eqb_d = din("eqb", (128, CC), F32)
    ivg2_d = din("ivg2", (128, CC), F32)
    ybc_d = din("ybc", (128, CC), F32)
    gyb_d = din("gyb", (128, CC), F32)
    be2c_d = din("be2c", (128, CC), F32)
    inv512_d = din("inv512", (128, 1), F32R)
    inv512b_d = din("inv512b", (128, 1), BF16)
    ones1x128_d = din("ones1x128", (1, 128), F32R)
    misc_d = din("misc", (128, 4), F32)
    outd = nc.dram_tensor("out", [C, L], BF16, kind="ExternalOutput").ap()

    z1qr = z1q_d.rearrange("(cc p) l -> p cc l", p=128)
    z2qr = z2q_d.rearrange("(cc p) l -> p cc l", p=128)
    z1rbr = z1rb_d.rearrange("(cc p) l -> p cc l", p=128)
    outr = outd.rearrange("(cc p) l -> p cc l", p=128)

    mm = nc.tensor.matmul
    tt = nc.vector.tensor_tensor
    ts = nc.vector.tensor_scalar
    stt = nc.vector.scalar_tensor_tensor
    act = nc.scalar.activation

    with tile.TileContext(nc) as tc, ExitStack() as ctx:
        cpool = ctx.enter_context(tc.tile_pool(name="consts", bufs=1))

        def const_tile(shape, dtype, src, tag):
            t = cpool.tile(list(shape), dtype, tag=tag, name=tag)
            nc.sync.dma_start(t[:], src)
            return t

        # phase-1 weights + small consts first (so phase 1 starts fast)
        wq8 = const_tile((128, CC, 512), F8, wq8_d, "wq8")
        wkv8 = const_tile((128, CC, 1024), F8, wkv8_d, "wkv8")
        eqb = const_tile((128, CC), F32, eqb_d, "eqb")
        misc = const_tile((128, 4), F32, misc_d, "misc")

        # phase-2-only consts: tiles now, DMAs deferred into the phase-1 loop
        def const_tile_nodma(shape, dtype, tag):
            return cpool.tile(list(shape), dtype, tag=tag, name=tag)

        wr8 = const_tile_nodma((128, CC, 512), F8, "wr8")
        selq = const_tile_nodma((128, CC, 8), F8, "selq")
        selbc = const_tile_nodma((8, CC, 128), F32R, "selbc")
        ivg2 = const_tile_nodma((128, CC), F32, "ivg2")
        ybc = const_tile_nodma((128, CC), F32, "ybc")
        gyb = const_tile_nodma((128, CC), F32, "gyb")
        be2c = const_tile_nodma((128, CC), F32, "be2c")
        inv512 = const_tile_nodma((128, 1), F32R, "inv512")
        inv512b = const_tile_nodma((128, 1), BF16, "inv512b")
        ones1x128 = const_tile_nodma((1, 128), F32R, "ones1x128")
        u1wa = const_tile_nodma((1, 1024), F32R, "u1wa")
        if not cst["w1bb0"] > 0.5:
            u1wb = const_tile_nodma((1, 1024), F32R, "u1wb")
            ones512 = const_tile_nodma((1, 512), F32R, "ones512")
        u2c = const_tile_nodma((128, 8), F32R, "u2c")
        g3g = const_tile_nodma((1, 512), F32R, "g3g")
        g3be = const_tile_nodma((1, 512), F32R, "g3be")

        def load_deferred_consts():
            nc.sync.dma_start(wr8[:], wr8_d)
            nc.sync.dma_start(selq[:], selq_d)
            nc.sync.dma_start(selbc[:], selbc_d)
            nc.sync.dma_start(ivg2[:], ivg2_d)
            nc.sync.dma_start(ybc[:], ybc_d)
            nc.sync.dma_start(gyb[:], gyb_d)
            nc.sync.dma_start(be2c[:], be2c_d)
            nc.sync.dma_start(inv512[:], inv512_d)
            nc.sync.dma_start(inv512b[:], inv512b_d)
            nc.sync.dma_start(ones1x128[:], ones1x128_d)
            nc.sync.dma_start(u1wa[:], u1wa_d)
            if not cst["w1bb0"] > 0.5:
                nc.sync.dma_start(u1wb[:], u1wb_d)
                nc.sync.dma_start(ones512[:], ones512_d)
            nc.sync.dma_start(u2c[:], u2c_d)
            nc.sync.dma_start(g3g[:], g3g_d)
            nc.sync.dma_start(g3be[:], g3be_d)

        # big FFN weights: DMA issued inside the phase-1 loop (overlap)
        w1g = cpool.tile([128, CC, 1024], F32R, tag="w1g", name="w1g")
        w2c = cpool.tile([128, 8, 512], F32R, tag="w2c", name="w2c")

        # persistent across phases
        qsm8 = cpool.tile([128, CC, L], F8, tag="qsm8", name="qsm8")
        ctxbd = [cpool.tile([128, 128], F8, tag=f"ctxbd{p}", name=f"ctxbd{p}")
                 for p in range(CC)]
        nc.scalar.add_instruction(mybir.InstLoadActFuncSet(
            name=nc.get_next_instruction_name(), ins=[], outs=[],
            act_func_set_id=6))

        # ---------------- Phase 1: q/k/v + exp + ctx accumulation ----------------
        with ExitStack() as p1:
            lp1 = p1.enter_context(tc.tile_pool(name="lp1", bufs=2))
            ps_kv = p1.enter_context(tc.tile_pool(name="ps_kv", bufs=2, space="PSUM"))
            ps_q = p1.enter_context(tc.tile_pool(name="ps_q", bufs=2, space="PSUM"))
            ps_ctx = p1.enter_context(tc.tile_pool(name="ps_ctx", bufs=1, space="PSUM"))

            ctxps = [ps_ctx.tile([128, 2, 129], F32, tag=f"ctx{i}", name=f"ctxps{i}")
                     for i in range(2)]

            pend = None       # lagged ctx MM emission: (ek8, v8t, is_first)
            first_ctx = True

            def flush_ctx(last=False):
                nonlocal pend, first_ctx
                if pend is None:
                    return
                ek8_, v8t_, _ = pend
                for s01 in range(2):
                    for pr in range(CC):
                        mm(ctxps[pr // 2][:, pr % 2, :],
                           ek8_[:, s01, pr * 128:(pr + 1) * 128],
                           v8t_[:, s01, pr, :],
                           start=(first_ctx and s01 == 0),
                           stop=(last and s01 == 1),
                           skip_group_check=True)
                first_ctx = False
                pend = None

            for t in range(NT):
                sl = slice(t * 512, (t + 1) * 512)
                z1t = lp1.tile([128, CC, 512], F8, tag="z1t")
                nc.sync.dma_start(z1t[:], z1qr[:, :, sl])
                z2t = lp1.tile([128, CC, 512], F8, tag="z2t")
                nc.sync.dma_start(z2t[:], z2qr[:, :, sl])
                # stream the big FFN weights during phase 1
                if t == 1:
                    nc.sync.dma_start(w1g[:], w1g_d)
                elif t == 2:
                    nc.sync.dma_start(w2c[:], w2c_d)
                elif t == 3:
                    load_deferred_consts()

                # ---- q in [o, l] ----
                if cst["bq0"] > 0.5:
                    # joint exp over oc-pairs (uniform bias ln(s_q))
                    for op_ in range(2):
                        qps = ps_q.tile([128, 2, 512], F32, tag="qpp",
                                        bufs=1, name="qps")
                        for i in range(2):
                            oc = op_ * 2 + i
                            for p in range(2):
                                mm(qps[:, i, :],
                                   wq8[:, 2 * p:2 * p + 2,
                                       oc * 128:(oc + 1) * 128],
                                   z1t[:, 2 * p:2 * p + 2, :],
                                   start=(p == 0), stop=(p == 1), perf_mode=DR)
                        act(qsm8[:, op_ * 2:op_ * 2 + 2, sl], qps[:], AFT.Exp,
                            bias=misc[:, 2:3], scale=cst["c_q"])
                else:
                    for oc in range(CC):
                        qps = ps_q.tile([128, 512], F32, tag="qps", name="qps")
                        for p in range(2):
                            mm(qps[:],
                               wq8[:, 2 * p:2 * p + 2, oc * 128:(oc + 1) * 128],
                               z1t[:, 2 * p:2 * p + 2, :],
                               start=(p == 0), stop=(p == 1), perf_mode=DR)
                        act(qsm8[:, oc, sl], qps[:], AFT.Exp,
                            bias=eqb[:, oc:oc + 1], scale=cst["c_q"])

                # ---- k/v per 128-l subtile; ctx lagged one sub-pair ----
                for sp in range(2):
                    ek8 = lp1.tile([128, 2, 512], F8, tag="ek8")
                    v8t = lp1.tile([128, 2, CC, 129], F8, tag="v8t")
                    nc.vector.memset(v8t[:, :, :, 128:129], 1.0)
                    kpair = ps_kv.tile([128, 2, 512], F32, tag="kpair",
                                       bufs=1, name="kpair")
                    for s01 in range(2):
                        sub = sp * 2 + s01
                        ssl = slice(sub * 128, (sub + 1) * 128)
                        vps = ps_kv.tile([128, 512], F32, tag="vps", name="vps")
                        for p in range(2):
                            mm(kpair[:, s01, :],
                               z2t[:, 2 * p:2 * p + 2, ssl],
                               wkv8[:, 2 * p:2 * p + 2, 0:512],
                               start=(p == 0), stop=(p == 1), perf_mode=DR)
                        for p in range(2):
                            mm(vps[:],
                               z2t[:, 2 * p:2 * p + 2, ssl],
                               wkv8[:, 2 * p:2 * p + 2, 512:1024],
                               start=(p == 0), stop=(p == 1), perf_mode=DR)
                        ts(v8t[:, s01, :, 0:128],
                           vps[:].rearrange("p (cc x) -> p cc x", x=128),
                           cst["c_v"], None, AluOpType.mult)
                    act(ek8[:], kpair[:], AFT.Exp,
                        bias=misc[:, 0:1], scale=cst["c_k"])
                    flush_ctx()
                    pend = (ek8, v8t, None)
            flush_ctx(last=True)

            # ---- finalize ctx: normalize rows, build block-diagonal fp8 ----
            rsc = lp1.tile([128, 2, 2], F32, tag="rsc", bufs=1)
            for i in range(2):
                nc.vector.reciprocal(rsc[:, i, :], ctxps[i][:, :, 128])
            for pr in range(CC):
                nc.vector.memset(ctxbd[pr][:], 0.0)
                for hb in range(2):
                    hs = slice(hb * 64, (hb + 1) * 64)
                    ts(ctxbd[pr][hs, hs],
                       ctxps[pr // 2][hs, pr % 2, hs],
                       rsc[hs, pr // 2, pr % 2:pr % 2 + 1],
                       cst["s_cv"], AluOpType.mult, AluOpType.mult)

        # ---------------- Phase 2: apply + reprojection + LN/FFN ----------------
        with ExitStack() as p2:
            lp2 = p2.enter_context(tc.tile_pool(name="lp2", bufs=2))
            lph = p2.enter_context(tc.tile_pool(name="lph", bufs=2))
            ps_t = p2.enter_context(tc.tile_pool(name="ps_t", bufs=2, space="PSUM"))
            ps_fp = p2.enter_context(tc.tile_pool(name="ps_fp", bufs=2, space="PSUM"))
            ps_f2 = p2.enter_context(tc.tile_pool(name="ps_f2", bufs=2, space="PSUM"))
            ps_row = p2.enter_context(tc.tile_pool(name="ps_row", bufs=1, space="PSUM"))

            def stage_A(t):
                sl = slice(t * 512, (t + 1) * 512)
                z1res = lp2.tile([128, CC, 512], BF16, tag="z1res", name="z1res")
                nc.sync.dma_start(z1res[:], z1rbr[:, :, sl])
                sqp = ps_row.tile([8, 512], F32, tag="aux", name="sqp")

                # q softmax denominators
                for pr in range(CC):
                    mm(sqp[:], selq[:, pr, :], qsm8[:, pr, sl],
                       start=(pr == 0), stop=(pr == CC - 1), skip_group_check=True)
                rq = lp2.tile([8, 512], F32R, tag="rq", name="rq")
                with nc.allow_low_precision(reason="rq rounds to f32r for bcast matmul"):
                    nc.vector.reciprocal(rq[:], sqp[:])

                # attention apply
                att8 = lp2.tile([128, CC, 512], F8, tag="att8", name="att8")
                rqbs = lp2.tile([128, CC, 512], BF16, tag="rqbs", bufs=1,
                                name="rqbs")
                for pr in range(CC):
                    rqb = ps_t.tile([128, 512], F32, tag="pst", name="rqb")
                    mm(rqb[:], selbc[:, pr, :], rq[:],
                       start=True, stop=True)
                    act(rqbs[:, pr, :], rqb[:], AFT.Copy)
                    attps = ps_t.tile([128, 512], F32, tag="pst", name="attps")
                    mm(attps[:], ctxbd[pr][:], qsm8[:, pr, sl],
                       start=True, stop=True)
                    tt(att8[:, pr, :], attps[:], rqbs[:, pr, :], AluOpType.mult)

                # z = Wr att + z1 + biases
                zt = lp2.tile([128, CC, 512], F32R, tag="zt", bufs=1, name="zt")
                zsq = lp2.tile([128, CC, 512], BF16, tag="zsq", bufs=1, name="zsq")
                for oc in range(CC):
                    zps = ps_t.tile([128, 512], F32, tag="pst", name="zps")
                    for p in range(2):
                        mm(zps[:], wr8[:, 2 * p:2 * p + 2, oc * 128:(oc + 1) * 128],
                           att8[:, 2 * p:2 * p + 2, :],
                           start=(p == 0), stop=(p == 1), perf_mode=DR)
                    stt(zt[:, oc, :], zps[:], cst["c_z"],
                        z1res[:, oc, :], AluOpType.mult, AluOpType.add)
                    act(zsq[:, oc, :], zt[:, oc, :].bitcast(F32), AFT.Square)
                # LN1 stats: mu then e2 through one rotating psum bank
                mu1p = ps_row.tile([1, 512], F32, tag="rowacc", name="mu1p")
                for oc in range(CC):
                    mm(mu1p[:], inv512[:], zt[:, oc, :],
                       start=(oc == 0), stop=(oc == CC - 1), skip_group_check=True)
                mu1row = lp2.tile([1, 512], F32, tag="mu1row", name="mu1row")
                act(mu1row[:], mu1p[:], AFT.Copy)
                e2p = ps_row.tile([1, 512], F32, tag="rowacc", name="e2p")
                for oc in range(CC):
                    mm(e2p[:], inv512b[:], zsq[:, oc, :],
                       start=(oc == 0), stop=(oc == CC - 1), skip_group_check=True)

                # LN1 rows: rsig1 = exp(-0.5 ln(var+eps))
                musq1 = lp2.tile([1, 512], F32, tag="musq1", name="musq1")
                act(musq1[:], mu1row[:], AFT.Square)
                var1 = lp2.tile([1, 512], F32, tag="var1", name="var1")
                tt(var1[:], e2p[:], musq1[:], AluOpType.subtract)
                lnv1 = lp2.tile([1, 512], F32, tag="lnv1", name="lnv1")
                act(lnv1[:], var1[:], AFT.Ln, bias=misc[0:1, 1:2])
                rs1 = lp2.tile([1, 512], F32R, tag="rs1", name="rs1")
                act(rs1[:], lnv1[:], AFT.Exp, scale=-0.5)
                mneg = lp2.tile([1, 512], F32R, tag="mneg", name="mneg")
                tt(mneg[:], mu1row[:], rs1[:], AluOpType.mult)
                rs1b = ps_row.tile([128, 512], F32, tag="aux", name="rs1b")
                mm(rs1b[:], ones1x128[:], rs1[:], start=True, stop=True,
                   skip_group_check=True)

                # zn = z * rsig1
                zn = lp2.tile([128, CC, 512], F32R, tag="zn", name="zn")
                for oc in range(CC):
                    tt(zn[:, oc, :], zt[:, oc, :].bitcast(F32), rs1b[:],
                       AluOpType.mult)
                return sl, zn, mneg

            def stage_B(st):
                sl, zn, mneg = st

                # FFN1 + ELU
                he2 = lph.tile([128, 8, 512], F32R, tag="he2", bufs=1, name="he2")
                for j in range(8):
                    js = slice(j * 128, (j + 1) * 128)
                    fps = ps_fp.tile([128, 512], F32, tag="fps", name="fps")
                    for cc2 in range(CC):
                        mm(fps[:], w1g[:, cc2, js], zn[:, cc2, :],
                           start=(cc2 == 0), stop=False)
                    mm(fps[:], u1wa[:, js], mneg[:], start=False,
                       stop=cst["w1bb0"] > 0.5, skip_group_check=True)
                    if not cst["w1bb0"] > 0.5:
                        mm(fps[:], u1wb[:, js], ones512[:], start=False,
                           stop=True, skip_group_check=True)
                    E = lp2.tile([128, 512], BF16, tag="E", bufs=2, name="E")
                    act(E[:], fps[:], AFT.Exp)
                    t2 = lp2.tile([128, 512], BF16, tag="t2", bufs=2, name="t2")
                    nc.gpsimd.tensor_scalar(t2[:], E[:], 1.0, None, AluOpType.min)
                    stt(he2[:, j, :], fps[:], 0.0, t2[:], AluOpType.max,
                        AluOpType.add)
                # LN2 mean rows (u2c . he2)
                mu2p = ps_row.tile([1, 512], F32, tag="rowacc", name="mu2p")
                for j in range(8):
                    mm(mu2p[:], u2c[:, j:j + 1], he2[:, j, :],
                       start=(j == 0), stop=(j == 7), skip_group_check=True)
                mu2row = lp2.tile([1, 512], F32, tag="mu2row", name="mu2row")
                act(mu2row[:], mu2p[:], AFT.Copy)
                e2yp = ps_row.tile([1, 512], F32, tag="rowacc", name="e2yp")

                # FFN2 o-outer + LN2 stats
                g2y = lph.tile([128, CC, 512], F32R, tag="g2y", bufs=2, name="g2y")
                for o2 in range(CC):
                    os_ = slice(o2 * 128, (o2 + 1) * 128)
                    f2ps = ps_f2.tile([128, 512], F32, tag="f2", name="f2ps")
                    for j in range(8):
                        mm(f2ps[:], w2c[:, j, os_], he2[:, j, :],
                           start=(j == 0), stop=(j == 7))
                    ysq = lp2.tile([128, 512], BF16, tag="ysq", name="ysq")
                    act(ysq[:], f2ps[:], AFT.Square,
                        bias=ybc[:, o2:o2 + 1], scale=ivg2[:, o2:o2 + 1])
                    mm(e2yp[:], inv512b[:], ysq[:],
                       start=(o2 == 0), stop=(o2 == CC - 1), skip_group_check=True)
                    act(g2y[:, o2, :], f2ps[:], AFT.Identity,
                        bias=gyb[:, o2:o2 + 1])

                # LN2 rows
                negmu2 = lp2.tile([1, 512], F32R, tag="negmu2", name="negmu2")
                ts(negmu2[:], mu2row[:], cst["kc"], -1.0,
                   AluOpType.add, AluOpType.mult)
                musq2 = lp2.tile([1, 512], F32, tag="musq2", name="musq2")
                tt(musq2[:], negmu2[:].bitcast(F32), negmu2[:].bitcast(F32),
                   AluOpType.mult)
                var2 = lp2.tile([1, 512], F32, tag="var2", name="var2")
                tt(var2[:], e2yp[:], musq2[:], AluOpType.subtract)
                lnv2 = lp2.tile([1, 512], F32, tag="lnv2", name="lnv2")
                act(lnv2[:], var2[:], AFT.Ln, bias=misc[0:1, 1:2])
                rs2 = lp2.tile([1, 512], F32R, tag="rs2", name="rs2")
                act(rs2[:], lnv2[:], AFT.Exp, scale=-0.5)
                sig2 = lp2.tile([1, 512], F32R, tag="sig2", name="sig2")
                act(sig2[:], lnv2[:], AFT.Exp, scale=0.5)
                rs2b = ps_row.tile([128, 512], F32, tag="aux", name="rs2b")
                mm(rs2b[:], ones1x128[:], rs2[:], start=True, stop=True,
                   skip_group_check=True)

                # finalize: out = (g2y + g2(x)negmu2 + be2(x)sig2) * rsig2
                outt = lp2.tile([128, CC, 512], BF16, tag="outt", bufs=1, name="outt")
                for o2 in range(CC):
                    os_ = slice(o2 * 128, (o2 + 1) * 128)
                    Rp = ps_t.tile([128, 512], F32, tag="pst", name="Rp")
                    mm(Rp[:], g3g[:, os_], nmrs[:], start=True, stop=True)
                    w_ = lp2.tile([128, 512], F32, tag="w_", name="w_")
                    tt(w_[:], g2y[:, o2, :].bitcast(F32), rs2b[:], AluOpType.mult)
                    stt(outt[:, o2, :], w_[:], be2c[:, o2:o2 + 1], Rp[:],
                        AluOpType.add, AluOpType.add)
                nc.sync.dma_start(outr[:, :, sl], outt[:])

            stA = stage_A(0)
            for t in range(1, NT):
                stA_next = stage_A(t)
                stage_B(stA)
                stA = stA_next
            stage_B(stA)

    nc.compile()
    return nc


def _prep_consts(z1, z2, Wq, bq, Wk, bk, Wv, bv, Wr, br, g1, be1,
                 W1, b1, W2, b2, g2, be2):
    import ml_dtypes
    f = np.float32
    F8NP = ml_dtypes.float8_e4m3

    def wscale(W):
        m = float(np.abs(W).max())
        return 160.0 / m if m > 0 else 1.0

    s1 = 160.0 / max(float(np.abs(z1).max()), 1e-30)
    s2 = 160.0 / max(float(np.abs(z2).max()), 1e-30)
    swq, swk, swv, swr = wscale(Wq), wscale(Wk), wscale(Wv), wscale(Wr)
    s_q, s_e, s_v, s_c, s_a = 4.0, 4.0, 32.0, 32.0, 32.0

    def chunkT(a, s):          # [out, in] -> [128, in//128, out] scaled fp8
        o, cin = a.shape
        return np.ascontiguousarray(
            (a.T * s).reshape(cin // 128, 128, o).transpose(1, 0, 2)).astype(F8NP)

    wq8 = chunkT(Wq, swq)                                       # [128, 4, 512]
    wkv = np.concatenate([Wk.T * swk, Wv.T * swv], axis=1)      # [512, 1024]
    wkv8 = np.ascontiguousarray(
        wkv.reshape(CC, 128, 1024).transpose(1, 0, 2)).astype(F8NP)
    wr8 = chunkT(Wr, swr)                                       # [128, 4, 512]

    W1g = (W1 * g1[None, :]).astype(f)                          # [1024, 512]
    w1g = np.ascontiguousarray(
        W1g.T.reshape(CC, 128, 1024).transpose(1, 0, 2)).astype(f)
    u1 = W1g.sum(axis=1).astype(f)
    w1bb = (W1 @ be1 + b1).astype(f)
    u1wa = (-u1).reshape(1, 1024).astype(f)
    u1wb = w1bb.reshape(1, 1024).astype(f)

    W2g = (W2 * g2[:, None]).astype(f)                          # [512, 1024]
    w2c = np.ascontiguousarray(
        W2g.T.reshape(8, 128, 512).transpose(1, 0, 2)).astype(f)
    u2 = (W2.sum(axis=0) / 512.0).astype(f)                     # [1024]
    u2c = np.ascontiguousarray(u2.reshape(8, 128).T).astype(f)  # [128, 8]
    w2sum = W2.sum(axis=1).astype(f)                            # [512]
    kc = float(np.mean(b2) - np.mean(w2sum))
    g3g = g2.reshape(1, 512).astype(f)
    g3be = be2.reshape(1, 512).astype(f)

    def colsC(v):
        return np.ascontiguousarray(np.asarray(v, f).reshape(CC, 128).T)

    selq = np.zeros((128, CC, 8), dtype=F8NP)
    selbc = np.zeros((8, CC, 128), dtype=f)
    for pr in range(CC):
        for hb in range(2):
            h = 2 * pr + hb
            selq[hb * 64:(hb + 1) * 64, pr, h] = 1.0
            selbc[h, pr, hb * 64:(hb + 1) * 64] = s_a / s_c

    consts = {
        "wq8": wq8, "wkv8": wkv8, "wr8": wr8,
        "w1g": w1g, "u1wa": u1wa, "u1wb": u1wb, "w2c": w2c, "u2c": u2c,
        "g3g": g3g, "g3be": g3be,
        "ones512": np.ones((1, 512), dtype=f),
        "selq": selq, "selbc": selbc,
        "eqb": colsC(bq + np.log(s_q)),
        "ivg2": colsC(1.0 / g2),
        "ybc": colsC(b2 - w2sum),
        "gyb": colsC(g2 * (b2 - w2sum)),
        "be2c": colsC(be2),
        "inv512": np.full((128, 1), 1.0 / 512.0, dtype=f),
        "inv512b": np.full((128, 1), 1.0 / 512.0, dtype=np.dtype("bfloat16") if False else f).astype(__import__("ml_dtypes").bfloat16),
        "ones1x128": np.ones((1, 128), dtype=f),
        "misc": np.stack([np.full(128, np.log(s_e), dtype=f), np.full(128, EPS, dtype=f), np.full(128, np.log(s_q), dtype=f), np.full(128, kc, dtype=f)], axis=1),
    }
    scalars = {
        "c_q": 1.0 / (swq * s1),
        "c_k": 1.0 / (swk * s2),
        "ln_se": float(np.log(s_e)),
        "c_v": s_v / (swv * s2),
        "s_cv": s_c / s_v,
        "c_z": 1.0 / (swr * s_a),
        "kc": kc,
        "w1bb0": 1.0 if not np.any(w1bb) else 0.0,
        "bq0": 1.0 if not np.any(bq) else 0.0,
    }
    import ml_dtypes as mld
    z1rb = (z1 + (br + Wr @ bv)[None, :, None]).astype(mld.bfloat16)
    return consts, scalars, z1rb, s1, s2


def kernel(**inputs):
    global LAST_RESULT, CONSTS
    import ml_dtypes
    F8NP = ml_dtypes.float8_e4m3
    z1 = np.asarray(inputs["z1"], dtype=np.float32)
    z2 = np.asarray(inputs["z2"], dtype=np.float32)
    wargs = [np.asarray(inputs[k], dtype=np.float32) for k in
             ["Wq", "bq", "Wk", "bk", "Wv", "bv", "Wr", "br", "g1", "be1",
              "W1", "b1", "W2", "b2", "g2", "be2"]]
    consts, scalars, z1rb, s1, s2 = _prep_consts(z1, z2, *wargs)

    key = ("prog",) + tuple(sorted((k, round(v * 1e9)) for k, v in scalars.items()))
    if key not in _CACHE:
        CONSTS = scalars
        _CACHE.clear()
        _CACHE[key] = _build_program()
    nc = _CACHE[key]

    z1q = np.ascontiguousarray((z1 * s1).astype(F8NP))
    z2q = np.ascontiguousarray((z2 * s2).astype(F8NP))

    in_maps = []
    for b in range(B):
        m = dict(consts)
        m["z1q"] = z1q[b]
        m["z2q"] = z2q[b]
        m["z1rb"] = np.ascontiguousarray(z1rb[b])
        in_maps.append(m)

    import os
    trace = bool(int(os.environ.get("KERNEL_TRACE", "0")))
    res = run_bass_kernel_spmd(nc, in_maps, list(range(B)), trace=trace)
    LAST_RESULT = res
    out = np.stack([np.asarray(res.results[b]["out"], dtype=np.float32) for b in range(B)], axis=0)
    return out


# revision 41
# speedup vs baseline: 2.2915x; 1.0019x over previous
"""CACombiner Trainium2 kernel: conv-projected efficient attention + FFN.

Data-parallel over batch: 8 batch elements -> 8 NeuronCores, identical SPMD
program per core. Attention block (q/k/v, ctx, apply, reprojection) runs in
fp8e4 with DoubleRow matmuls; the FFN runs in float32r. LayerNorm rsqrt is
computed as exp(-0.5*ln(var+eps)) so every activation stays in one ACT table.
"""
import sys
sys.path.insert(0, "/opt/trn_rl_repo")
from contextlib import ExitStack

import numpy as np

import concourse.bass as bass
import concourse.tile as tile
from concourse import mybir, bacc
from concourse.bass_utils import run_bass_kernel_spmd
from concourse.alu_op_type import AluOpType

F32 = mybir.dt.float32
F32R = mybir.dt.float32r
BF16 = mybir.dt.bfloat16
F8 = mybir.dt.float8e4
AFT = mybir.ActivationFunctionType
DR = mybir.MatmulPerfMode.DoubleRow

B, C, L = 8, 512, 4096
H, DK = 8, 64
EPS = 1e-5
CC = C // 128           # 4 channel chunks
NT = L // 512           # 8 l-tiles (512 wide)

_CACHE = {}
LAST_RESULT = None

# compile-time floats baked into the program (set before _build_program)
CONSTS = {}


def _build_program():
    cst = CONSTS
    nc = bacc.Bacc("TRN2", target_bir_lowering=False, debug=False)

    def din(name, shape, dtype):
        return nc.dram_tensor(name, list(shape), dtype, kind="ExternalInput").ap()

    z1q_d = din("z1q", (C, L), F8)
    z2q_d = din("z2q", (C, L), F8)
    z1rb_d = din("z1rb", (C, L), BF16)
    wq8_d = din("wq8", (128, CC, 512), F8)
    wkv8_d = din("wkv8", (128, CC, 1024), F8)
    wr8_d = din("wr8", (128, CC, 512), F8)
    w1g_d = din("w1g", (128, CC, 1024), F32R)
    u1wa_d = din("u1wa", (1, 1024), F32R)
    u1wb_d = din("u1wb", (1, 1024), F32R)
    w2c_d = din("w2c", (128, 8, 512), F32R)
    u2c_d = din("u2c", (128, 8), F32R)
    g3g_d = din("g3g", (1, 512), F32R)
    g3be_d = din("g3be", (1, 512), F32R)
    ones512_d = din("ones512", (1, 512), F32R)
    selq_d = din("selq", (128, CC, 8), F8)
    selbc_d = din("selbc", (8, CC, 128), F32R)
    eqb_d = din("eqb", (128, CC), F32)
    ivg2_d = din("ivg2", (128, CC), F32)
    ybc_d = din("ybc", (128, CC), F32)
    gyb_d = din("gyb", (128, CC), F32)
    be2c_d = din("be2c", (128, CC), F32)
    inv512_d = din("inv512", (128, 1), F32R)
    inv512b_d = din("inv512b", (128, 1), BF16)
    ones1x128_d = din("ones1x128", (1, 128), F32R)
    misc_d = din("misc", (128, 4), F32)
    outd = nc.dram_tensor("out", [C, L], BF16, kind="ExternalOutput").ap()

    z1qr = z1q_d.rearrange("(cc p) l -> p cc l", p=128)
    z2qr = z2q_d.rearrange("(cc p) l -> p cc l", p=128)
    z1rbr = z1rb_d.rearrange("(cc p) l -> p cc l", p=128)
    outr = outd.rearrange("(cc p) l -> p cc l", p=128)

    mm = nc.tensor.matmul
    tt = nc.vector.tensor_tensor
    ts = nc.vector.tensor_scalar
    stt = nc.vector.scalar_tensor_tensor
    act = nc.scalar.activation

    with tile.TileContext(nc) as tc, ExitStack() as ctx:
        cpool = ctx.enter_context(tc.tile_pool(name="consts", bufs=1))

        def const_tile(shape, dtype, src, tag):
            t = cpool.tile(list(shape), dtype, tag=tag, name=tag)
            nc.sync.dma_start(t[:], src)
            return t

        # phase-1 weights + small consts first (so phase 1 starts fast)
        wq8 = const_tile((128, CC, 512), F8, wq8_d, "wq8")
        wkv8 = const_tile((128, CC, 1024), F8, wkv8_d, "wkv8")
        eqb = const_tile((128, CC), F32, eqb_d, "eqb")
        misc = const_tile((128, 4), F32, misc_d, "misc")

        # phase-2-only consts: tiles now, DMAs deferred into the phase-1 loop
        def const_tile_nodma(shape, dtype, tag):
            return cpool.tile(list(shape), dtype, tag=tag, name=tag)

        wr8 = const_tile_nodma((128, CC, 512), F8, "wr8")
        selq = const_tile_nodma((128, CC, 8), F8, "selq")
        selbc = const_tile_nodma((8, CC, 128), F32R, "selbc")
        ivg2 = const_tile_nodma((128, CC), F32, "ivg2")
        ybc = const_tile_nodma((128, CC), F32, "ybc")
        gyb = const_tile_nodma((128, CC), F32, "gyb")
        be2c = const_tile_nodma((128, CC), F32, "be2c")
        inv512 = const_tile_nodma((128, 1), F32R, "inv512")
        inv512b = const_tile_nodma((128, 1), BF16, "inv512b")
        ones1x128 = const_tile_nodma((1, 128), F32R, "ones1x128")
        u1wa = const_tile_nodma((1, 1024), F32R, "u1wa")
        if not cst["w1bb0"] > 0.5:
            u1wb = const_tile_nodma((1, 1024), F32R, "u1wb")
            ones512 = const_tile_nodma((1, 512), F32R, "ones512")
        u2c = const_tile_nodma((128, 8), F32R, "u2c")
        g3g = const_tile_nodma((1, 512), F32R, "g3g")
        g3be = const_tile_nodma((1, 512), F32R, "g3be")

        def load_deferred_consts():
            nc.sync.dma_start(wr8[:], wr8_d)
            nc.sync.dma_start(selq[:], selq_d)
            nc.sync.dma_start(selbc[:], selbc_d)
            nc.sync.dma_start(ivg2[:], ivg2_d)
            nc.sync.dma_start(ybc[:], ybc_d)
            nc.sync.dma_start(gyb[:], gyb_d)
            nc.sync.dma_start(be2c[:], be2c_d)
            nc.sync.dma_start(inv512[:], inv512_d)
            nc.sync.dma_start(inv512b[:], inv512b_d)
            nc.sync.dma_start(ones1x128[:], ones1x128_d)
            nc.sync.dma_start(u1wa[:], u1wa_d)
            if not cst["w1bb0"] > 0.5:
                nc.sync.dma_start(u1wb[:], u1wb_d)
                nc.sync.dma_start(ones512[:], ones512_d)
            nc.sync.dma_start(u2c[:], u2c_d)
            nc.sync.dma_start(g3g[:], g3g_d)
            nc.sync.dma_start(g3be[:], g3be_d)

        # big FFN weights: DMA issued inside the phase-1 loop (overlap)
        w1g = cpool.tile([128, CC, 1024], F32R, tag="w1g", name="w1g")
        w2c = cpool.tile([128, 8, 512], F32R, tag="w2c", name="w2c")

        # persistent across phases
        qsm8 = cpool.tile([128, CC, L], F8, tag="qsm8", name="qsm8")
        ctxbd = [cpool.tile([128, 128], F8, tag=f"ctxbd{p}", name=f"ctxbd{p}")
                 for p in range(CC)]
        nc.scalar.add_instruction(mybir.InstLoadActFuncSet(
            name=nc.get_next_instruction_name(), ins=[], outs=[],
            act_func_set_id=6))

        # ---------------- Phase 1: q/k/v + exp + ctx accumulation ----------------
        with ExitStack() as p1:
            lp1 = p1.enter_context(tc.tile_pool(name="lp1", bufs=2))
            ps_kv = p1.enter_context(tc.tile_pool(name="ps_kv", bufs=2, space="PSUM"))
            ps_q = p1.enter_context(tc.tile_pool(name="ps_q", bufs=2, space="PSUM"))
            ps_ctx = p1.enter_context(tc.tile_pool(name="ps_ctx", bufs=1, space="PSUM"))

            ctxps = [ps_ctx.tile([128, 2, 129], F32, tag=f"ctx{i}", name=f"ctxps{i}")
                     for i in range(2)]

            pend = None       # lagged ctx MM emission: (ek8, v8t, is_first)
            first_ctx = True

            def flush_ctx(last=False):
                nonlocal pend, first_ctx
                if pend is None:
                    return
                ek8_, v8t_, _ = pend
                for s01 in range(2):
                    for pr in range(CC):
                        mm(ctxps[pr // 2][:, pr % 2, :],
                           ek8_[:, s01, pr * 128:(pr + 1) * 128],
                           v8t_[:, s01, pr, :],
                           start=(first_ctx and s01 == 0),
                           stop=(last and s01 == 1),
                           skip_group_check=True)
                first_ctx = False
                pend = None

            for t in range(NT):
                sl = slice(t * 512, (t + 1) * 512)
                z1t = lp1.tile([128, CC, 512], F8, tag="z1t")
                nc.sync.dma_start(z1t[:], z1qr[:, :, sl])
                z2t = lp1.tile([128, CC, 512], F8, tag="z2t")
                nc.sync.dma_start(z2t[:], z2qr[:, :, sl])
                # stream the big FFN weights during phase 1
                if t == 1:
                    nc.sync.dma_start(w1g[:], w1g_d)
                elif t == 2:
                    nc.sync.dma_start(w2c[:], w2c_d)
                elif t == 3:
                    load_deferred_consts()

                # ---- q in [o, l] ----
                if cst["bq0"] > 0.5:
                    # joint exp over oc-pairs (uniform bias ln(s_q))
                    for op_ in range(2):
                        qps = ps_q.tile([128, 2, 512], F32, tag="qpp",
                                        bufs=1, name="qps")
                        for i in range(2):
                            oc = op_ * 2 + i
                            for p in range(2):
                                mm(qps[:, i, :],
                                   wq8[:, 2 * p:2 * p + 2,
                                       oc * 128:(oc + 1) * 128],
                                   z1t[:, 2 * p:2 * p + 2, :],
                                   start=(p == 0), stop=(p == 1), perf_mode=DR)
                        act(qsm8[:, op_ * 2:op_ * 2 + 2, sl], qps[:], AFT.Exp,
                            bias=misc[:, 2:3], scale=cst["c_q"])
                else:
                    for oc in range(CC):
                        qps = ps_q.tile([128, 512], F32, tag="qps", name="qps")
                        for p in range(2):
                            mm(qps[:],
                               wq8[:, 2 * p:2 * p + 2, oc * 128:(oc + 1) * 128],
                               z1t[:, 2 * p:2 * p + 2, :],
                               start=(p == 0), stop=(p == 1), perf_mode=DR)
                        act(qsm8[:, oc, sl], qps[:], AFT.Exp,
                            bias=eqb[:, oc:oc + 1], scale=cst["c_q"])

                # ---- k/v per 128-l subtile; ctx lagged one sub-pair ----
                for sp in range(2):
                    ek8 = lp1.tile([128, 2, 512], F8, tag="ek8")
                    v8t = lp1.tile([128, 2, CC, 129], F8, tag="v8t")
                    nc.vector.memset(v8t[:, :, :, 128:129], 1.0)
                    kpair = ps_kv.tile([128, 2, 512], F32, tag="kpair",
                                       bufs=1, name="kpair")
                    for s01 in range(2):
                        sub = sp * 2 + s01
                        ssl = slice(sub * 128, (sub + 1) * 128)
                        vps = ps_kv.tile([128, 512], F32, tag="vps", name="vps")
                        for p in range(2):
                            mm(kpair[:, s01, :],
                               z2t[:, 2 * p:2 * p + 2, ssl],
                               wkv8[:, 2 * p:2 * p + 2, 0:512],
                               start=(p == 0), stop=(p == 1), perf_mode=DR)
                        for p in range(2):
                            mm(vps[:],
                               z2t[:, 2 * p:2 * p + 2, ssl],
                               wkv8[:, 2 * p:2 * p + 2, 512:1024],
                               start=(p == 0), stop=(p == 1), perf_mode=DR)
                        ts(v8t[:, s01, :, 0:128],
                           vps[:].rearrange("p (cc x) -> p cc x", x=128),
                           cst["c_v"], None, AluOpType.mult)
                    act(ek8[:], kpair[:], AFT.Exp,
                        bias=misc[:, 0:1], scale=cst["c_k"])
                    flush_ctx()
                    pend = (ek8, v8t, None)
            flush_ctx(last=True)

            # ---- finalize ctx: normalize rows, build block-diagonal fp8 ----
            rsc = lp1.tile([128, 2, 2], F32, tag="rsc", bufs=1)
            for i in range(2):
                nc.vector.reciprocal(rsc[:, i, :], ctxps[i][:, :, 128])
            for pr in range(CC):
                nc.vector.memset(ctxbd[pr][:], 0.0)
                for hb in range(2):
                    hs = slice(hb * 64, (hb + 1) * 64)
                    ts(ctxbd[pr][hs, hs],
                       ctxps[pr // 2][hs, pr % 2, hs],
                       rsc[hs, pr // 2, pr % 2:pr % 2 + 1],
                       cst["s_cv"], AluOpType.mult, AluOpType.mult)

        # ---------------- Phase 2: apply + reprojection + LN/FFN ----------------
        with ExitStack() as p2:
            lp2 = p2.enter_context(tc.tile_pool(name="lp2", bufs=2))
            lph = p2.enter_context(tc.tile_pool(name="lph", bufs=2))
            ps_t = p2.enter_context(tc.tile_pool(name="ps_t", bufs=2, space="PSUM"))
            ps_fp = p2.enter_context(tc.tile_pool(name="ps_fp", bufs=2, space="PSUM"))
            ps_f2 = p2.enter_context(tc.tile_pool(name="ps_f2", bufs=2, space="PSUM"))
            ps_row = p2.enter_context(tc.tile_pool(name="ps_row", bufs=1, space="PSUM"))

            def stage_A(t):
                sl = slice(t * 512, (t + 1) * 512)
                z1res = lp2.tile([128, CC, 512], BF16, tag="z1res", name="z1res")
                nc.sync.dma_start(z1res[:], z1rbr[:, :, sl])
                sqp = ps_row.tile([8, 512], F32, tag="aux", name="sqp")

                # q softmax denominators
                for pr in range(CC):
                    mm(sqp[:], selq[:, pr, :], qsm8[:, pr, sl],
                       start=(pr == 0), stop=(pr == CC - 1), skip_group_check=True)
                rq = lp2.tile([8, 512], F32R, tag="rq", name="rq")
                with nc.allow_low_precision(reason="rq rounds to f32r for bcast matmul"):
                    nc.vector.reciprocal(rq[:], sqp[:])

                # attention apply
                att8 = lp2.tile([128, CC, 512], F8, tag="att8", name="att8")
                rqbs = lp2.tile([128, CC, 512], BF16, tag="rqbs", bufs=1,
                                name="rqbs")
                for pr in range(CC):
                    rqb = ps_t.tile([128, 512], F32, tag="pst", name="rqb")
                    mm(rqb[:], selbc[:, pr, :], rq[:],
                       start=True, stop=True)
                    act(rqbs[:, pr, :], rqb[:], AFT.Copy)
                    attps = ps_t.tile([128, 512], F32, tag="pst", name="attps")
                    mm(attps[:], ctxbd[pr][:], qsm8[:, pr, sl],
                       start=True, stop=True)
                    tt(att8[:, pr, :], attps[:], rqbs[:, pr, :], AluOpType.mult)

                # z = Wr att + z1 + biases
                zt = lp2.tile([128, CC, 512], F32R, tag="zt", bufs=1, name="zt")
                zsq = lp2.tile([128, CC, 512], BF16, tag="zsq", bufs=1, name="zsq")
                for oc in range(CC):
                    zps = ps_t.tile([128, 512], F32, tag="pst", name="zps")
                    for p in range(2):
                        mm(zps[:], wr8[:, 2 * p:2 * p + 2, oc * 128:(oc + 1) * 128],
                           att8[:, 2 * p:2 * p + 2, :],
                           start=(p == 0), stop=(p == 1), perf_mode=DR)
                    stt(zt[:, oc, :], zps[:], cst["c_z"],
                        z1res[:, oc, :], AluOpType.mult, AluOpType.add)
                    act(zsq[:, oc, :], zt[:, oc, :].bitcast(F32), AFT.Square)
                # LN1 stats: mu then e2 through one rotating psum bank
                mu1p = ps_row.tile([1, 512], F32, tag="rowacc", name="mu1p")
                for oc in range(CC):
                    mm(mu1p[:], inv512[:], zt[:, oc, :],
                       start=(oc == 0), stop=(oc == CC - 1), skip_group_check=True)
                mu1row = lp2.tile([1, 512], F32, tag="mu1row", name="mu1row")
                act(mu1row[:], mu1p[:], AFT.Copy)
                e2p = ps_row.tile([1, 512], F32, tag="rowacc", name="e2p")
                for oc in range(CC):
                    mm(e2p[:], inv512b[:], zsq[:, oc, :],
                       start=(oc == 0), stop=(oc == CC - 1), skip_group_check=True)

                # LN1 rows: rsig1 = exp(-0.5 ln(var+eps))
                musq1 = lp2.tile([1, 512], F32, tag="musq1", name="musq1")
                act(musq1[:], mu1row[:], AFT.Square)
                var1 = lp2.tile([1, 512], F32, tag="var1", name="var1")
                tt(var1[:], e2p[:], musq1[:], AluOpType.subtract)
                lnv1 = lp2.tile([1, 512], F32, tag="lnv1", name="lnv1")
                act(lnv1[:], var1[:], AFT.Ln, bias=misc[0:1, 1:2])
                rs1 = lp2.tile([1, 512], F32R, tag="rs1", name="rs1")
                act(rs1[:], lnv1[:], AFT.Exp, scale=-0.5)
                mneg = lp2.tile([1, 512], F32R, tag="mneg", name="mneg")
                tt(mneg[:], mu1row[:], rs1[:], AluOpType.mult)
                rs1b = ps_row.tile([128, 512], F32, tag="aux", name="rs1b")
                mm(rs1b[:], ones1x128[:], rs1[:], start=True, stop=True,
                   skip_group_check=True)

                # zn = z * rsig1
                zn = lp2.tile([128, CC, 512], F32R, tag="zn", name="zn")
                for oc in range(CC):
                    tt(zn[:, oc, :], zt[:, oc, :].bitcast(F32), rs1b[:],
                       AluOpType.mult)
                return sl, zn, mneg

            def stage_B(st):
                sl, zn, mneg = st

                # FFN1 + ELU
                he2 = lph.tile([128, 8, 512], F32R, tag="he2", bufs=1, name="he2")
                for j in range(8):
                    js = slice(j * 128, (j + 1) * 128)
                    fps = ps_fp.tile([128, 512], F32, tag="fps", name="fps")
                    for cc2 in range(CC):
                        mm(fps[:], w1g[:, cc2, js], zn[:, cc2, :],
                           start=(cc2 == 0), stop=False)
                    mm(fps[:], u1wa[:, js], mneg[:], start=False,
                       stop=cst["w1bb0"] > 0.5, skip_group_check=True)
                    if not cst["w1bb0"] > 0.5:
                        mm(fps[:], u1wb[:, js], ones512[:], start=False,
                           stop=True, skip_group_check=True)
                    E = lp2.tile([128, 512], BF16, tag="E", bufs=2, name="E")
                    act(E[:], fps[:], AFT.Exp)
                    t2 = lp2.tile([128, 512], BF16, tag="t2", bufs=2, name="t2")
                    nc.gpsimd.tensor_scalar(t2[:], E[:], 1.0, None, AluOpType.min)
                    stt(he2[:, j, :], fps[:], 0.0, t2[:], AluOpType.max,
                        AluOpType.add)
                # LN2 mean rows (u2c . he2)
                mu2p = ps_row.tile([1, 512], F32, tag="rowacc", name="mu2p")
                for j in range(8):
                    mm(mu2p[:], u2c[:, j:j + 1], he2[:, j, :],
                       start=(j == 0), stop=(j == 7), skip_group_check=True)
                mu2row = lp2.tile([1, 512], F32, tag="mu2row", name="mu2row")
                act(mu2row[:], mu2p[:], AFT.Copy)
                e2yp = ps_row.tile([1, 512], F32, tag="rowacc", name="e2yp")

                # FFN2 o-outer + LN2 stats
                g2y = lph.tile([128, CC, 512], F32R, tag="g2y", bufs=2, name="g2y")
                for o2 in range(CC):
                    os_ = slice(o2 * 128, (o2 + 1) * 128)
                    f2ps = ps_f2.tile([128, 512], F32, tag="f2", name="f2ps")
                    for j in range(8):
                        mm(f2ps[:], w2c[:, j, os_], he2[:, j, :],
                           start=(j == 0), stop=(j == 7))
                    ysq = lp2.tile([128, 512], BF16, tag="ysq", name="ysq")
                    act(ysq[:], f2ps[:], AFT.Square,
                        bias=ybc[:, o2:o2 + 1], scale=ivg2[:, o2:o2 + 1])
                    mm(e2yp[:], inv512b[:], ysq[:],
                       start=(o2 == 0), stop=(o2 == CC - 1), skip_group_check=True)
                    act(g2y[:, o2, :], f2ps[:], AFT.Identity,
                        bias=gyb[:, o2:o2 + 1])

                # LN2 rows
                negmu2 = lp2.tile([1, 512], F32R, tag="negmu2", name="negmu2")
                ts(negmu2[:], mu2row[:], cst["kc"], -1.0,
                   AluOpType.add, AluOpType.mult)
                musq2 = lp2.tile([1, 512], F32, tag="musq2", name="musq2")
                tt(musq2[:], negmu2[:].bitcast(F32), negmu2[:].bitcast(F32),
                   AluOpType.mult)
                var2 = lp2.tile([1, 512], F32, tag="var2", name="var2")
                tt(var2[:], e2yp[:], musq2[:], AluOpType.subtract)
                lnv2 = lp2.tile([1, 512], F32, tag="lnv2", name="lnv2")
                act(lnv2[:], var2[:], AFT.Ln, bias=misc[0:1, 1:2])
                rs2 = lp2.tile([1, 512], F32R, tag="rs2", name="rs2")
                act(rs2[:], lnv2[:], AFT.Exp, scale=-0.5)
                sig2 = lp2.tile([1, 512], F32R, tag="sig2", name="sig2")
                act(sig2[:], lnv2[:], AFT.Exp, scale=0.5)
                rs2b = ps_row.tile([128, 512], F32, tag="aux", name="rs2b")
                mm(rs2b[:], ones1x128[:], rs2[:], start=True, stop=True,
                   skip_group_check=True)

                # finalize: out = (g2y + g2(x)negmu2 + be2(x)sig2) * rsig2
                outt = lp2.tile([128, CC, 512], BF16, tag="outt", bufs=1, name="outt")
                if cst["g2u"] > 0.5:
                    Rp0 = ps_t.tile([128, 512], F32, tag="pst", name="Rp0")
                    mm(Rp0[:], g3g[:, 0:128], nmrs[:], start=True, stop=True)
                for o2 in range(CC):
                    os_ = slice(o2 * 128, (o2 + 1) * 128)
                    if cst["g2u"] > 0.5:
                        Rp = Rp0
                    else:
                        Rp = ps_t.tile([128, 512], F32, tag="pst", name="Rp")
                        mm(Rp[:], g3g[:, os_], nmrs[:], start=True, stop=True)
                    w_ = lp2.tile([128, 512], F32, tag="w_", name="w_")
                    tt(w_[:], g2y[:, o2, :].bitcast(F32), rs2b[:], AluOpType.mult)
                    stt(outt[:, o2, :], w_[:], be2c[:, o2:o2 + 1], Rp[:],
                        AluOpType.add, AluOpType.add)
                nc.sync.dma_start(outr[:, :, sl], outt[:])

            stA = stage_A(0)
            for t in range(1, NT):
                stA_next = stage_A(t)
                stage_B(stA)
                stA = stA_next
            stage_B(stA)

    nc.compile()
    return nc


def _prep_consts(z1, z2, Wq, bq, Wk, bk, Wv, bv, Wr, br, g1, be1,
                 W1, b1, W2, b2, g2, be2):
    import ml_dtypes
    f = np.float32
    F8NP = ml_dtypes.float8_e4m3

    def wscale(W):
        m = float(np.abs(W).max())
        return 160.0 / m if m > 0 else 1.0

    s1 = 160.0 / max(float(np.abs(z1).max()), 1e-30)
    s2 = 160.0 / max(float(np.abs(z2).max()), 1e-30)
    swq, swk, swv, swr = wscale(Wq), wscale(Wk), wscale(Wv), wscale(Wr)
    s_q, s_e, s_v, s_c, s_a = 4.0, 4.0, 32.0, 32.0, 32.0

    def chunkT(a, s):          # [out, in] -> [128, in//128, out] scaled fp8
        o, cin = a.shape
        return np.ascontiguousarray(
            (a.T * s).reshape(cin // 128, 128, o).transpose(1, 0, 2)).astype(F8NP)

    wq8 = chunkT(Wq, swq)                                       # [128, 4, 512]
    wkv = np.concatenate([Wk.T * swk, Wv.T * swv], axis=1)      # [512, 1024]
    wkv8 = np.ascontiguousarray(
        wkv.reshape(CC, 128, 1024).transpose(1, 0, 2)).astype(F8NP)
    wr8 = chunkT(Wr, swr)                                       # [128, 4, 512]

    W1g = (W1 * g1[None, :]).astype(f)                          # [1024, 512]
    w1g = np.ascontiguousarray(
        W1g.T.reshape(CC, 128, 1024).transpose(1, 0, 2)).astype(f)
    u1 = W1g.sum(axis=1).astype(f)
    w1bb = (W1 @ be1 + b1).astype(f)
    u1wa = (-u1).reshape(1, 1024).astype(f)
    u1wb = w1bb.reshape(1, 1024).astype(f)

    W2g = (W2 * g2[:, None]).astype(f)                          # [512, 1024]
    w2c = np.ascontiguousarray(
        W2g.T.reshape(8, 128, 512).transpose(1, 0, 2)).astype(f)
    u2 = (W2.sum(axis=0) / 512.0).astype(f)                     # [1024]
    u2c = np.ascontiguousarray(u2.reshape(8, 128).T).astype(f)  # [128, 8]
    w2sum = W2.sum(axis=1).astype(f)                            # [512]
    kc = float(np.mean(b2) - np.mean(w2sum))
    g3g = g2.reshape(1, 512).astype(f)
    g3be = be2.reshape(1, 512).astype(f)

    def colsC(v):
        return np.ascontiguousarray(np.asarray(v, f).reshape(CC, 128).T)

    selq = np.zeros((128, CC, 8), dtype=F8NP)
    selbc = np.zeros((8, CC, 128), dtype=f)
    for pr in range(CC):
        for hb in range(2):
            h = 2 * pr + hb
            selq[hb * 64:(hb + 1) * 64, pr, h] = 1.0
            selbc[h, pr, hb * 64:(hb + 1) * 64] = s_a / s_c

    consts = {
        "wq8": wq8, "wkv8": wkv8, "wr8": wr8,
        "w1g": w1g, "u1wa": u1wa, "u1wb": u1wb, "w2c": w2c, "u2c": u2c,
        "g3g": g3g, "g3be": g3be,
        "ones512": np.ones((1, 512), dtype=f),
        "selq": selq, "selbc": selbc,
        "eqb": colsC(bq + np.log(s_q)),
        "ivg2": colsC(1.0 / g2),
        "ybc": colsC(b2 - w2sum),
        "gyb": colsC(g2 * (b2 - w2sum)),
        "be2c": colsC(be2),
        "inv512": np.full((128, 1), 1.0 / 512.0, dtype=f),
        "inv512b": np.full((128, 1), 1.0 / 512.0, dtype=np.dtype("bfloat16") if False else f).astype(__import__("ml_dtypes").bfloat16),
        "ones1x128": np.ones((1, 128), dtype=f),
        "misc": np.stack([np.full(128, np.log(s_e), dtype=f), np.full(128, EPS, dtype=f), np.full(128, np.log(s_q), dtype=f), np.full(128, kc, dtype=f)], axis=1),
    }
    scalars = {
        "c_q": 1.0 / (swq * s1),
        "c_k": 1.0 / (swk * s2),
        "ln_se": float(np.log(s_e)),
        "c_v": s_v / (swv * s2),
        "s_cv": s_c / s_v,
        "c_z": 1.0 / (swr * s_a),
        "kc": kc,
        "w1bb0": 1.0 if not np.any(w1bb) else 0.0,
        "bq0": 1.0 if not np.any(bq) else 0.0,
        "g2u": 1.0 if np.all(g2 == g2[0]) else 0.0,
    }
    import ml_dtypes as mld
    z1rb = (z1 + (br + Wr @ bv)[None, :, None]).astype(mld.bfloat16)
    return consts, scalars, z1rb, s1, s2


def kernel(**inputs):
    global LAST_RESULT, CONSTS
    import ml_dtypes
    F8NP = ml_dtypes.float8_e4m3
    z1 = np.asarray(inputs["z1"], dtype=np.float32)
    z2 = np.asarray(inputs["z2"], dtype=np.float32)
    wargs = [np.asarray(inputs[k], dtype=np.float32) for k in
             ["Wq", "bq", "Wk", "bk", "Wv", "bv", "Wr", "br", "g1", "be1",
              "W1", "b1", "W2", "b2", "g2", "be2"]]
    consts, scalars, z1rb, s1, s2 = _prep_consts(z1, z2, *wargs)

    key = ("prog",) + tuple(sorted((k, round(v * 1e9)) for k, v in scalars.items()))
    if key not in _CACHE:
        CONSTS = scalars
        _CACHE.clear()
        _CACHE[key] = _build_program()
    nc = _CACHE[key]

    z1q = np.ascontiguousarray((z1 * s1).astype(F8NP))
    z2q = np.ascontiguousarray((z2 * s2).astype(F8NP))

    in_maps = []
    for b in range(B):
        m = dict(consts)
        m["z1q"] = z1q[b]
        m["z2q"] = z2q[b]
        m["z1rb"] = np.ascontiguousarray(z1rb[b])
        in_maps.append(m)

    import os
    trace = bool(int(os.environ.get("KERNEL_TRACE", "0")))
    res = run_bass_kernel_spmd(nc, in_maps, list(range(B)), trace=trace)
    LAST_RESULT = res
    out = np.stack([np.asarray(res.results[b]["out"], dtype=np.float32) for b in range(B)], axis=0)
    return out
